# revision 13
# baseline (speedup 1.0000x reference)
"""Trainium2 Bass kernel for nn_EquivariantDeepSetsEncoder.

Strategy: data-parallel over batch (B=8) across 8 NeuronCores; one batch per
core. Per core the full 2048x2048 attention matrix E = exp(-pairwise_dist)
stays resident in SBUF (8 MB in bf16) and is reused by all three
message-passing layers, so HBM traffic is just the tiny inputs/outputs.

Key algebraic simplifications (exact, not approximations):
  * pairwise distances depend only on coordinate differences, so the centroid
    subtraction cancels inside the attention; E is built from raw x.
  * -dist(i,j) = x_i.x_j - |x_i|^2/2 - |x_j|^2/2 (then exp with scale=2),
    which a single augmented matmul computes.
  * softmax row-normalization (1/rowsum) cancels through LayerNorm's scale
    invariance because the pre-LN bias b_i is zero, so it is never computed.
  * LayerNorm mean comes for free as an extra (negated, pre-averaged) column
    of the layer weight matrix.

Precision: fp32 matmuls stream at 4 cycles/column on the PE, bf16 at 1. The
attention logits need ~fp32 accuracy, so they use a hi/lo split-precision
bf16 matmul (x = xh + xl, keeping the xh*xh + xh*xl + xl*xh terms, same for
the squared norms; K grows 5 -> 13 but moving cycles drop 4x). E, h and W
are plain bf16 (their quantization error averages out over 2048 points);
LayerNorm statistics and activations are computed in fp32.
"""

import math
import os

import numpy as np
import ml_dtypes

import concourse.bass as bass
import concourse.bacc as bacc
import concourse.mybir as mybir
import concourse.tile as tile
from concourse.bass_utils import run_bass_kernel_spmd
from concourse.vector_clock import ScopedClock

F32 = mybir.dt.float32
BF16 = mybir.dt.bfloat16
U32 = mybir.dt.uint32
AF = mybir.ActivationFunctionType
OP = mybir.AluOpType

B, N, D = 8, 2048, 3
P, R = 128, 16          # N = P * R; point (p, r) = original index 16*p + r
HID = (64, 128, 256)
LAT = 128
EPS = 1e-6
RSQRT_MAGIC = 0x5F3759DF


# ---------------------------------------------------------------------------
# Workaround for a walrus codegen limit in this toolchain: a NO_STRUCT
# instruction (Drain) can carry at most one sync-wait command. Tile's exit
# path attaches the full global-clock wait set to a single drain; split the
# waits across several drains instead. (Bacc's later legalization handles
# the rest of the instructions.)
def _split_drain_and_barrier(self, tick_clock, wait_clock):
    nc = self.nc
    drain_inst = nc.sync.drain()
    wait_clock.add_sem_waits(
        drain_inst.ins, ScopedClock({None: tick_clock.global_clock})
    )
    si = drain_inst.ins.sync_info
    waits = list(si.on_wait) if si is not None else []
    if len(waits) > 1:
        si.on_wait = [waits[0]]
        for w in waits[1:]:
            d2 = nc.sync.drain()
            if d2.ins.sync_info is not None:
                d2.ins.sync_info.on_wait = [w]
            else:
                d2.ins.sync_info = mybir.SyncInfo(on_wait=[w], on_update=[])
    nc.all_engine_barrier()
    assert self.sems is not None
    popped = nc._tile_sem_poison_stack.pop()
    assert popped is self._sem_poison
    nc.clear_and_free_semaphores(list(self.sems.allocated().values()))
    nc.all_engine_barrier()


def _apply_tile_patch():
    if os.environ.get("NO_DRAIN_PATCH", "0") == "1":
        return
    tile.TileContext._drain_and_barrier = _split_drain_and_barrier


# ---------------------------------------------------------------------------
def _emit_rsqrt(nc, out_ap, var_ap, w_t, t1_t, d_out):
    """out = sqrt(d_out / (var_ap + d_out*EPS)) == 1/sqrt(var + EPS), where
    var_ap holds sum-of-squares (d_out * var). Fast-inverse-sqrt seed plus
    three Newton iterations, fp32-accurate; DVE only (no activation table)."""
    nc.vector.tensor_single_scalar(out=w_t, in_=var_ap, scalar=d_out * EPS, op=OP.add)
    w_u = w_t.bitcast(U32)
    t1_u = t1_t.bitcast(U32)
    # seed bits = MAGIC - (w_bits >> 1). The DVE ALU is fp32 for +/-, so the
    # subtraction happens in float on the integer VALUES (result stays in
    # [5e8, 1.6e9], no wrap) and the uint32 output cast restores the bits;
    # the ~1e-5 relative bit noise is far below the seed's 3.4% error.
    nc.vector.tensor_scalar(
        out=t1_u, in0=w_u, scalar1=1, scalar2=None, op0=OP.logical_shift_right,
    )
    r_t = out_ap
    r_u = r_t.bitcast(U32)
    nc.vector.tensor_scalar(
        out=r_u, in0=t1_u, scalar1=-1.0, scalar2=float(RSQRT_MAGIC),
        op0=OP.mult, op1=OP.add,
    )
    sqd = math.sqrt(float(d_out))
    for it in range(3):
        nc.vector.tensor_tensor(out=t1_t, in0=r_t, in1=r_t, op=OP.mult)
        nc.vector.tensor_tensor(out=t1_t, in0=t1_t, in1=w_t, op=OP.mult)
        nc.vector.tensor_scalar(
            out=t1_t, in0=t1_t, scalar1=-0.5, scalar2=1.5, op0=OP.mult, op1=OP.add
        )
        if it < 2:
            nc.vector.tensor_tensor(out=r_t, in0=r_t, in1=t1_t, op=OP.mult)
        else:
            # fold the sqrt(d_out) factor into the final Newton multiply
            nc.vector.scalar_tensor_tensor(
                out=r_t, in0=r_t, scalar=sqd, in1=t1_t, op0=OP.mult, op1=OP.mult
            )


def _build(reps=1):
    nc = bacc.Bacc()
    xt = nc.dram_tensor("xt", [D, N], F32, kind="ExternalInput")
    xb = nc.dram_tensor("xb", [P, R * D], F32, kind="ExternalInput")
    mk = nc.dram_tensor("mk", [P, R], F32, kind="ExternalInput")
    w0a = nc.dram_tensor("w0a", [D, 2 * (HID[0] + 1)], BF16, kind="ExternalInput")
    w1a = nc.dram_tensor("w1a", [HID[0], 2 * (HID[1] + 1)], BF16, kind="ExternalInput")
    w2a = nc.dram_tensor("w2a", [HID[1], 2 * (HID[2] + 1)], BF16, kind="ExternalInput")
    wz2 = nc.dram_tensor("wz2", [P, 4 * LAT], BF16, kind="ExternalInput")
    bzt = nc.dram_tensor("bzt", [P, 1], F32, kind="ExternalInput")
    zout = nc.dram_tensor("z", [P, 1], F32, kind="ExternalOutput")
    cent_d = nc.dram_tensor("cent_scratch", [1, D], F32, kind="Internal")

    with tile.TileContext(nc) as tc:
        with tc.tile_pool(name="persist", bufs=1) as pp, \
             tc.tile_pool(name="scr", bufs=2) as scr:
            E_all = pp.tile([P, R * N], BF16, name="E_all")
            U13 = pp.tile([13, N], BF16, name="U13")
            V13 = pp.tile([13, N], BF16, name="V13")
            xtf = pp.tile([D, N], F32, name="xtf")
            xh3 = pp.tile([D, N], BF16, name="xh3")
            xl3 = pp.tile([D, N], BF16, name="xl3")
            xsq = pp.tile([D, N], F32, name="xsq")
            sneg = pp.tile([1, N], F32, name="sneg")
            shl = pp.tile([1, N], BF16, name="shl")
            sll = pp.tile([1, N], BF16, name="sll")
            onesb2 = pp.tile([2, N], BF16, name="onesb2")
            h0 = pp.tile([P, R * D], BF16, name="h0")
            h1 = pp.tile([P, R * HID[0]], BF16, name="h1")
            h2 = pp.tile([P, R * HID[1]], BF16, name="h2")
            h3 = pp.tile([P, R * HID[2]], BF16, name="h3")
            y_all = pp.tile([P, R * HID[2]], F32, name="y_all")
            EhT = pp.tile([P, N], BF16, name="EhT")
            xb_s = pp.tile([P, R * D], F32, name="xb_s")
            mk_s = pp.tile([P, R], F32, name="mk_s")
            msc = pp.tile([P, R], F32, name="msc")
            msc_b = pp.tile([P, R], BF16, name="msc_b")
            crow = pp.tile([1, R * D], F32, name="crow")
            w0_s = pp.tile([D, 2 * (HID[0] + 1)], BF16, name="w0_s")
            w1_s = pp.tile([HID[0], 2 * (HID[1] + 1)], BF16, name="w1_s")
            w2_s = pp.tile([HID[1], 2 * (HID[2] + 1)], BF16, name="w2_s")
            wz_s = pp.tile([P, 4 * LAT], BF16, name="wz_s")
            gfl_b = pp.tile([P, 2], BF16, name="gfl_b")
            bz_s = pp.tile([P, 1], F32, name="bz_s")
            ones31 = pp.tile([D, 1], F32, name="ones31")
            ones128 = pp.tile([P, 1], F32, name="ones128")
            ones1r = pp.tile([1, P], F32, name="ones1r")
            mkr = pp.tile([P, 1], F32, name="mkr")
            cnt_sb = pp.tile([1, 1], F32, name="cnt_sb")
            invc1 = pp.tile([1, 1], F32, name="invc1")
            invc_sb = pp.tile([P, 1], F32, name="invc_sb")
            cent_sb = pp.tile([D, 1], F32, name="cent_sb")
            varN = pp.tile([P, R], F32, name="varN")
            rstd = pp.tile([P, R], F32, name="rstd")
            rs_w = pp.tile([P, 8], F32, name="rs_w")
            rs_t1 = pp.tile([P, 8], F32, name="rs_t1")
            gf_b = pp.tile([P, 2], BF16, name="gf_b")
            z_sb = pp.tile([P, 1], F32, name="z_sb")

            for _rep in range(reps):
                # ------------- front: loads, hi/lo U/V build, centroid -------
                nc.sync.dma_start(out=xtf, in_=xt[:, :])
                nc.scalar.dma_start(out=xb_s, in_=xb[:, :])
                nc.scalar.dma_start(out=mk_s, in_=mk[:, :])
                nc.gpsimd.dma_start(out=w0_s, in_=w0a[:, :])
                nc.gpsimd.dma_start(out=w1_s, in_=w1a[:, :])
                nc.gpsimd.dma_start(out=w2_s, in_=w2a[:, :])
                nc.gpsimd.dma_start(out=wz_s, in_=wz2[:, :])
                nc.gpsimd.dma_start(out=bz_s, in_=bzt[:, :])
                nc.vector.memset(onesb2, 1.0)
                nc.gpsimd.memset(ones31, 1.0)
                nc.gpsimd.memset(ones128, 1.0)
                nc.gpsimd.memset(ones1r, 1.0)
                # hi/lo split of the coordinates
                nc.vector.tensor_copy(out=xh3, in_=xtf)
                nc.vector.tensor_tensor(out=xl3, in0=xtf, in1=xh3, op=OP.subtract)
                nc.scalar.activation(out=xsq, in_=xtf, func=AF.Square)

                with tc.tile_pool(name="fpsum", bufs=1, space="PSUM") as fp:
                    sqp = fp.tile([1, N], F32, name="sqp")
                    for g in range(4):
                        nc.tensor.matmul(
                            sqp[:, 512 * g:512 * (g + 1)], lhsT=ones31,
                            rhs=xsq[:, 512 * g:512 * (g + 1)], start=True, stop=True,
                        )
                    nc.vector.tensor_scalar_mul(out=sneg, in0=sqp, scalar1=-0.5)
                    nc.vector.tensor_copy(out=shl, in_=sneg)
                    nc.vector.tensor_tensor(out=sll, in0=sneg, in1=shl, op=OP.subtract)
                    # engines only address partition starts {0,32,64,96}; DMA
                    # places single rows at arbitrary partitions.
                    # U rows: xh xh xl | -sqh/2 -sql/2 | 1 1
                    # V rows: xh xl xh |   1     1     | -sqh/2 -sql/2
                    nc.scalar.dma_start(out=U13[0:3, :], in_=xh3)
                    nc.scalar.dma_start(out=U13[3:6, :], in_=xh3)
                    nc.scalar.dma_start(out=U13[6:9, :], in_=xl3)
                    nc.scalar.dma_start(out=U13[9:10, :], in_=shl)
                    nc.scalar.dma_start(out=U13[10:11, :], in_=sll)
                    nc.scalar.dma_start(out=U13[11:13, :], in_=onesb2)
                    nc.sync.dma_start(out=V13[0:3, :], in_=xh3)
                    nc.sync.dma_start(out=V13[3:6, :], in_=xl3)
                    nc.sync.dma_start(out=V13[6:9, :], in_=xh3)
                    nc.sync.dma_start(out=V13[9:11, :], in_=onesb2)
                    nc.sync.dma_start(out=V13[11:12, :], in_=shl)
                    nc.sync.dma_start(out=V13[12:13, :], in_=sll)

                    # centroid = sum(x*m)/max(count,1); count = sum(m)
                    nc.vector.reduce_sum(out=mkr, in_=mk_s, axis=mybir.AxisListType.X)
                    cntp = fp.tile([1, 1], F32, name="cntp")
                    nc.tensor.matmul(cntp, lhsT=mkr, rhs=ones128, start=True, stop=True)
                    nc.vector.tensor_scalar_max(out=cnt_sb, in0=cntp, scalar1=1.0)
                    nc.vector.reciprocal(out=invc1, in_=cnt_sb)
                    invb = fp.tile([P, 1], F32, name="invb")
                    nc.tensor.matmul(invb, lhsT=ones1r, rhs=invc1, start=True, stop=True)
                    nc.vector.tensor_copy(out=invc_sb, in_=invb)
                    nc.vector.tensor_scalar_mul(out=msc, in0=mk_s, scalar1=invc_sb)
                    nc.vector.tensor_copy(out=msc_b, in_=msc)
                    centp = fp.tile([D, 1], F32, name="centp")
                    for r in range(R):
                        nc.tensor.matmul(
                            centp, lhsT=xb_s[:, D * r:D * (r + 1)], rhs=msc[:, r:r + 1],
                            start=(r == 0), stop=(r == R - 1),
                        )
                    nc.vector.tensor_copy(out=cent_sb, in_=centp)
                    nc.gpsimd.dma_start(out=cent_d[:, :], in_=cent_sb)
                    cent_ap = cent_d[:, :]
                    cbc = bass.AP(
                        tensor=cent_ap.tensor, offset=cent_ap.offset,
                        ap=[[0, 1], [0, R], [1, D]],
                    )
                    nc.gpsimd.dma_start(out=crow, in_=cbc)
                    c48p = fp.tile([P, R * D], F32, name="c48p")
                    nc.tensor.matmul(c48p, lhsT=ones1r, rhs=crow, start=True, stop=True)
                    nc.vector.tensor_tensor(out=h0, in0=xb_s, in1=c48p, op=OP.subtract)

                # ------------- phases 1+2 share the PSUM budget --------------
                with tc.tile_pool(name="spsum", bufs=2, space="PSUM") as sp, \
                     tc.tile_pool(name="apsum", bufs=2, space="PSUM") as apl, \
                     tc.tile_pool(name="bpsum", bufs=2, space="PSUM") as bpl:
                    # phase 1: E = exp(-dist), 16 row-blocks of [128, 2048]
                    for i in range(R):
                        for t in range(2):
                            ps = sp.tile([P, 1024], F32, name="ps", tag="ps")
                            for gg in range(2):
                                j0 = 1024 * t + 512 * gg
                                nc.tensor.matmul(
                                    ps[:, 512 * gg:512 * (gg + 1)],
                                    lhsT=U13[:, P * i:P * (i + 1)],
                                    rhs=V13[:, j0:j0 + 512], start=True, stop=True,
                                )
                            nc.scalar.activation(
                                out=E_all[:, N * i + 1024 * t: N * i + 1024 * (t + 1)],
                                in_=ps, func=AF.Exp, scale=2.0,
                            )

                    # phase 2: three message-passing layers
                    layers = [
                        (h0, D, w0_s, HID[0], h1),
                        (h1, HID[0], w1_s, HID[1], h2),
                        (h2, HID[1], w2_s, HID[2], h3),
                    ]
                    for li, (hin, d_in, w_s, d_out, hout) in enumerate(layers):
                        # (E @ h)^T accumulated over the 16 point-chunks
                        for g in range(4):
                            pa = apl.tile([P, 512], F32, name="pa", tag="pa")
                            for r in range(R):
                                nc.tensor.matmul(
                                    pa[:d_in, :], lhsT=hin[:, d_in * r:d_in * (r + 1)],
                                    rhs=E_all[:, N * r + 512 * g: N * r + 512 * (g + 1)],
                                    start=(r == 0), stop=(r == R - 1),
                                )
                            nc.vector.tensor_copy(
                                out=EhT[:d_in, 512 * g:512 * (g + 1)], in_=pa[:d_in, :]
                            )
                        # @W_aug, center, variance, rsqrt, scale, swish
                        for half in range(2):
                            for c in range(8 * half, 8 * half + 8):
                                pb = bpl.tile([P, d_out + 1], F32, name="pb", tag="pb")
                                ehc = EhT[:d_in, P * c:P * (c + 1)]
                                nc.tensor.matmul(
                                    pb, lhsT=ehc, rhs=w_s[:, 0:d_out + 1],
                                    start=True, stop=False,
                                )
                                nc.tensor.matmul(
                                    pb, lhsT=ehc,
                                    rhs=w_s[:, d_out + 1:2 * (d_out + 1)],
                                    start=False, stop=True,
                                )
                                ysl = y_all[:, d_out * c:d_out * (c + 1)]
                                # y0 = u - mean(u)   (psum col d_out holds -mean)
                                nc.vector.tensor_scalar(
                                    out=ysl, in0=pb[:, :d_out],
                                    scalar1=pb[:, d_out:d_out + 1], scalar2=None,
                                    op0=OP.add,
                                )
                                sqo = scr.tile([P, d_out], F32, name="sqo", tag="sqo")
                                nc.scalar.activation(
                                    out=sqo, in_=ysl, func=AF.Square,
                                    accum_out=varN[:, c:c + 1],
                                )
                            h8 = slice(8 * half, 8 * half + 8)
                            _emit_rsqrt(
                                nc, rstd[:, h8], varN[:, h8], rs_w, rs_t1, d_out
                            )
                            for c in range(8 * half, 8 * half + 8):
                                ysl = y_all[:, d_out * c:d_out * (c + 1)]
                                nc.vector.tensor_scalar_mul(
                                    out=ysl, in0=ysl, scalar1=rstd[:, c:c + 1]
                                )
                            yhalf = y_all[:, d_out * 8 * half:d_out * 8 * (half + 1)]
                            hhalf = hout[:, d_out * 8 * half:d_out * 8 * (half + 1)]
                            nc.scalar.activation(out=hhalf, in_=yhalf, func=AF.Silu)

                # ------------- phase 3: masked mean pool + readout -----------
                with tc.tile_pool(name="tpsum", bufs=1, space="PSUM") as tp:
                    gf0 = tp.tile([P, 1], F32, name="gf0")
                    gf1 = tp.tile([P, 1], F32, name="gf1")
                    for t, gft in enumerate((gf0, gf1)):
                        for c in range(R):
                            o = HID[2] * c + P * t
                            nc.tensor.matmul(
                                gft, lhsT=h3[:, o:o + P], rhs=msc_b[:, c:c + 1],
                                start=(c == 0), stop=(c == R - 1),
                            )
                    nc.vector.tensor_copy(out=gf_b[:, 0:1], in_=gf0)
                    nc.vector.tensor_copy(out=gf_b[:, 1:2], in_=gf1)
                    nc.vector.tensor_tensor(out=gfl_b[:, 0:1], in0=gf0,
                                            in1=gf_b[:, 0:1], op=OP.subtract)
                    nc.vector.tensor_tensor(out=gfl_b[:, 1:2], in0=gf1,
                                            in1=gf_b[:, 1:2], op=OP.subtract)
                    zps = tp.tile([P, 1], F32, name="zps")
                    # wz_s columns: [wzh half0 | wzh half1 | wzl half0 | wzl half1]
                    # z ~= Wzh.gfh + Wzl.gfh + Wzh.gfl   (drop Wzl.gfl)
                    zmm = [(0, gf_b, 0), (1, gf_b, 1), (2, gf_b, 0), (3, gf_b, 1),
                           (0, gfl_b, 0), (1, gfl_b, 1)]
                    for k, (wcol, gsrc, gcol) in enumerate(zmm):
                        nc.tensor.matmul(
                            zps, lhsT=wz_s[:, LAT * wcol:LAT * (wcol + 1)],
                            rhs=gsrc[:, gcol:gcol + 1],
                            start=(k == 0), stop=(k == len(zmm) - 1),
                        )
                    nc.vector.scalar_tensor_tensor(
                        out=z_sb, in0=zps, scalar=1.0, in1=bz_s,
                        op0=OP.mult, op1=OP.add,
                    )
                    nc.sync.dma_start(out=zout[:, :], in_=z_sb)
    return nc


_NC_CACHE = None


def _get_nc():
    global _NC_CACHE
    if _NC_CACHE is None:
        _apply_tile_patch()
        nc = _build()
        nc.finalize()   # Bacc.compile(): wait legalization + register alloc
        _NC_CACHE = nc
    return _NC_CACHE


def _host_prep(inputs):
    x = np.asarray(inputs["x"], np.float32)
    mask = np.asarray(inputs["mask"], np.float32)
    W = [np.asarray(inputs[f"W{i}"], np.float32) for i in range(3)]
    Wz = np.asarray(inputs["Wz"], np.float32)
    bz = np.asarray(inputs["bz"], np.float32)

    def hilo(a):
        hi = a.astype(ml_dtypes.bfloat16)
        lo = (a - hi.astype(np.float32)).astype(ml_dtypes.bfloat16)
        return hi, lo

    waug = []
    for i in range(3):
        a = np.concatenate([W[i], -W[i].mean(axis=1, keepdims=True)], axis=1)
        hi, lo = hilo(a)
        waug.append(np.ascontiguousarray(np.concatenate([hi, lo], axis=1)))
    wzflat = np.concatenate([Wz[:P, :], Wz[P:, :]], axis=1)
    wzh, wzl = hilo(wzflat)
    wz2 = np.ascontiguousarray(np.concatenate([wzh, wzl], axis=1))
    bzr = np.ascontiguousarray(bz.reshape(P, 1))

    in_maps = []
    for bi in range(B):
        in_maps.append({
            "xt": np.ascontiguousarray(
                x[bi].reshape(P, R, D).transpose(2, 1, 0).reshape(D, N)
            ),
            "xb": np.ascontiguousarray(x[bi].reshape(P, R * D)),
            "mk": np.ascontiguousarray(mask[bi].reshape(P, R)),
            "w0a": waug[0], "w1a": waug[1], "w2a": waug[2],
            "wz2": wz2, "bzt": bzr,
        })
    return in_maps


def kernel(**inputs):
    for i in range(3):
        if (np.any(np.asarray(inputs[f"b{i}"])) or
                np.any(np.asarray(inputs[f"be{i}"])) or
                np.any(np.asarray(inputs[f"g{i}"]) != 1.0)):
            raise NotImplementedError(
                "kernel specialized for zero LN/layer biases and unit gains"
            )
    in_maps = _host_prep(inputs)
    nc = _get_nc()
    res = run_bass_kernel_spmd(nc, in_maps, core_ids=list(range(B)))
    return np.stack([res.results[i]["z"][:, 0] for i in range(B)]).astype(np.float32)


# revision 14
# speedup vs baseline: 1.1589x; 1.1589x over previous
"""Trainium2 Bass kernel for nn_EquivariantDeepSetsEncoder.

Strategy: data-parallel over batch (B=8) across 8 NeuronCores; one batch per
core. Per core the full 2048x2048 attention matrix E = exp(-pairwise_dist)
stays resident in SBUF (8 MB in bf16) and is reused by all three
message-passing layers, so HBM traffic is just the tiny inputs/outputs.

Key algebraic simplifications (exact, not approximations):
  * pairwise distances depend only on coordinate differences, so the centroid
    subtraction cancels inside the attention; E is built from raw x.
  * -dist(i,j) = x_i.x_j - |x_i|^2/2 - |x_j|^2/2 (then exp with scale=2),
    which a single augmented matmul computes.
  * softmax row-normalization (1/rowsum) cancels through LayerNorm's scale
    invariance because the pre-LN bias b_i is zero, so it is never computed.
  * LayerNorm mean comes for free as an extra (negated, pre-averaged) column
    of the layer weight matrix.

Precision: fp32 matmuls stream at 4 cycles/column on the PE, bf16 at 1. The
attention logits need ~fp32 accuracy, so they use a hi/lo split-precision
bf16 matmul (x = xh + xl, keeping the xh*xh + xh*xl + xl*xh terms, same for
the squared norms; K grows 5 -> 13 but moving cycles drop 4x). E, h and W
are plain bf16 (their quantization error averages out over 2048 points);
LayerNorm statistics and activations are computed in fp32.
"""

import math
import os

import numpy as np
import ml_dtypes

import concourse.bass as bass
import concourse.bacc as bacc
import concourse.mybir as mybir
import concourse.tile as tile
from concourse.bass_utils import run_bass_kernel_spmd
from concourse.vector_clock import ScopedClock

F32 = mybir.dt.float32
BF16 = mybir.dt.bfloat16
U32 = mybir.dt.uint32
AF = mybir.ActivationFunctionType
OP = mybir.AluOpType

B, N, D = 8, 2048, 3
P, R = 128, 16          # N = P * R; point (p, r) = original index 16*p + r
HID = (64, 128, 256)
LAT = 128
EPS = 1e-6
RSQRT_MAGIC = 0x5F3759DF


# ---------------------------------------------------------------------------
# Workaround for a walrus codegen limit in this toolchain: a NO_STRUCT
# instruction (Drain) can carry at most one sync-wait command. Tile's exit
# path attaches the full global-clock wait set to a single drain; split the
# waits across several drains instead. (Bacc's later legalization handles
# the rest of the instructions.)
def _split_drain_and_barrier(self, tick_clock, wait_clock):
    nc = self.nc
    drain_inst = nc.sync.drain()
    wait_clock.add_sem_waits(
        drain_inst.ins, ScopedClock({None: tick_clock.global_clock})
    )
    si = drain_inst.ins.sync_info
    waits = list(si.on_wait) if si is not None else []
    if len(waits) > 1:
        si.on_wait = [waits[0]]
        for w in waits[1:]:
            d2 = nc.sync.drain()
            if d2.ins.sync_info is not None:
                d2.ins.sync_info.on_wait = [w]
            else:
                d2.ins.sync_info = mybir.SyncInfo(on_wait=[w], on_update=[])
    nc.all_engine_barrier()
    assert self.sems is not None
    popped = nc._tile_sem_poison_stack.pop()
    assert popped is self._sem_poison
    nc.clear_and_free_semaphores(list(self.sems.allocated().values()))
    nc.all_engine_barrier()


def _apply_tile_patch():
    if os.environ.get("NO_DRAIN_PATCH", "0") == "1":
        return
    tile.TileContext._drain_and_barrier = _split_drain_and_barrier


# ---------------------------------------------------------------------------
def _emit_rsqrt(nc, out_ap, var_ap, w_t, t1_t, d_out):
    """out = sqrt(d_out / (var_ap + d_out*EPS)) == 1/sqrt(var + EPS), where
    var_ap holds sum-of-squares (d_out * var). Fast-inverse-sqrt seed plus
    three Newton iterations, fp32-accurate; DVE only (no activation table)."""
    nc.vector.tensor_single_scalar(out=w_t, in_=var_ap, scalar=d_out * EPS, op=OP.add)
    w_u = w_t.bitcast(U32)
    t1_u = t1_t.bitcast(U32)
    # seed bits = MAGIC - (w_bits >> 1). The DVE ALU is fp32 for +/-, so the
    # subtraction happens in float on the integer VALUES (result stays in
    # [5e8, 1.6e9], no wrap) and the uint32 output cast restores the bits;
    # the ~1e-5 relative bit noise is far below the seed's 3.4% error.
    nc.vector.tensor_scalar(
        out=t1_u, in0=w_u, scalar1=1, scalar2=None, op0=OP.logical_shift_right,
    )
    r_t = out_ap
    r_u = r_t.bitcast(U32)
    nc.vector.tensor_scalar(
        out=r_u, in0=t1_u, scalar1=-1.0, scalar2=float(RSQRT_MAGIC),
        op0=OP.mult, op1=OP.add,
    )
    sqd = math.sqrt(float(d_out))
    for it in range(2):
        nc.vector.tensor_tensor(out=t1_t, in0=r_t, in1=r_t, op=OP.mult)
        nc.vector.tensor_tensor(out=t1_t, in0=t1_t, in1=w_t, op=OP.mult)
        nc.vector.tensor_scalar(
            out=t1_t, in0=t1_t, scalar1=-0.5, scalar2=1.5, op0=OP.mult, op1=OP.add
        )
        if it < 1:
            nc.vector.tensor_tensor(out=r_t, in0=r_t, in1=t1_t, op=OP.mult)
        else:
            # fold the sqrt(d_out) factor into the final Newton multiply
            nc.vector.scalar_tensor_tensor(
                out=r_t, in0=r_t, scalar=sqd, in1=t1_t, op0=OP.mult, op1=OP.mult
            )


def _build(reps=1):
    nc = bacc.Bacc()
    xt = nc.dram_tensor("xt", [D, N], F32, kind="ExternalInput")
    xb = nc.dram_tensor("xb", [P, R * D], F32, kind="ExternalInput")
    mk = nc.dram_tensor("mk", [P, R], F32, kind="ExternalInput")
    w0a = nc.dram_tensor("w0a", [D, 2 * (HID[0] + 1)], BF16, kind="ExternalInput")
    w1a = nc.dram_tensor("w1a", [HID[0], 2 * (HID[1] + 1)], BF16, kind="ExternalInput")
    w2a = nc.dram_tensor("w2a", [HID[1], 2 * (HID[2] + 1)], BF16, kind="ExternalInput")
    wz2 = nc.dram_tensor("wz2", [P, 4 * LAT], BF16, kind="ExternalInput")
    bzt = nc.dram_tensor("bzt", [P, 1], F32, kind="ExternalInput")
    zout = nc.dram_tensor("z", [P, 1], F32, kind="ExternalOutput")
    cent_d = nc.dram_tensor("cent_scratch", [1, D], F32, kind="Internal")

    with tile.TileContext(nc) as tc:
        with tc.tile_pool(name="persist", bufs=1) as pp, \
             tc.tile_pool(name="scr", bufs=2) as scr:
            E_all = pp.tile([P, R * N], BF16, name="E_all")
            U13 = pp.tile([13, N], BF16, name="U13")
            V13 = pp.tile([13, N], BF16, name="V13")
            xtf = pp.tile([D, N], F32, name="xtf")
            xh3 = pp.tile([D, N], BF16, name="xh3")
            xl3 = pp.tile([D, N], BF16, name="xl3")
            xsq = pp.tile([D, N], F32, name="xsq")
            sneg = pp.tile([1, N], F32, name="sneg")
            shl = pp.tile([1, N], BF16, name="shl")
            sll = pp.tile([1, N], BF16, name="sll")
            onesb2 = pp.tile([2, N], BF16, name="onesb2")
            h0 = pp.tile([P, R * D], BF16, name="h0")
            h1 = pp.tile([P, R * HID[0]], BF16, name="h1")
            h2 = pp.tile([P, R * HID[1]], BF16, name="h2")
            h3 = pp.tile([P, R * HID[2]], BF16, name="h3")
            y_all = pp.tile([P, R * HID[2]], F32, name="y_all")
            EhT = pp.tile([P, N], BF16, name="EhT")
            xb_s = pp.tile([P, R * D], F32, name="xb_s")
            mk_s = pp.tile([P, R], F32, name="mk_s")
            msc = pp.tile([P, R], F32, name="msc")
            msc_b = pp.tile([P, R], BF16, name="msc_b")
            crow = pp.tile([1, R * D], F32, name="crow")
            w0_s = pp.tile([D, 2 * (HID[0] + 1)], BF16, name="w0_s")
            w1_s = pp.tile([HID[0], 2 * (HID[1] + 1)], BF16, name="w1_s")
            w2_s = pp.tile([HID[1], 2 * (HID[2] + 1)], BF16, name="w2_s")
            wz_s = pp.tile([P, 4 * LAT], BF16, name="wz_s")
            gfl_b = pp.tile([P, 2], BF16, name="gfl_b")
            bz_s = pp.tile([P, 1], F32, name="bz_s")
            ones31 = pp.tile([D, 1], F32, name="ones31")
            ones128 = pp.tile([P, 1], F32, name="ones128")
            ones1r = pp.tile([1, P], F32, name="ones1r")
            mkr = pp.tile([P, 1], F32, name="mkr")
            cnt_sb = pp.tile([1, 1], F32, name="cnt_sb")
            invc1 = pp.tile([1, 1], F32, name="invc1")
            invc_sb = pp.tile([P, 1], F32, name="invc_sb")
            cent_sb = pp.tile([D, 1], F32, name="cent_sb")
            varN = pp.tile([P, R], F32, name="varN")
            rstd = pp.tile([P, R], F32, name="rstd")
            rs_w = pp.tile([P, 8], F32, name="rs_w")
            rs_t1 = pp.tile([P, 8], F32, name="rs_t1")
            gf_b = pp.tile([P, 2], BF16, name="gf_b")
            z_sb = pp.tile([P, 1], F32, name="z_sb")

            for _rep in range(reps):
                # ------------- front: loads, hi/lo U/V build, centroid -------
                nc.sync.dma_start(out=xtf, in_=xt[:, :])
                nc.scalar.dma_start(out=xb_s, in_=xb[:, :])
                nc.scalar.dma_start(out=mk_s, in_=mk[:, :])
                nc.gpsimd.dma_start(out=w0_s, in_=w0a[:, :])
                nc.gpsimd.dma_start(out=w1_s, in_=w1a[:, :])
                nc.gpsimd.dma_start(out=w2_s, in_=w2a[:, :])
                nc.gpsimd.dma_start(out=wz_s, in_=wz2[:, :])
                nc.gpsimd.dma_start(out=bz_s, in_=bzt[:, :])
                nc.vector.memset(onesb2, 1.0)
                nc.gpsimd.memset(ones31, 1.0)
                nc.gpsimd.memset(ones128, 1.0)
                nc.gpsimd.memset(ones1r, 1.0)
                # hi/lo split of the coordinates
                nc.vector.tensor_copy(out=xh3, in_=xtf)
                nc.vector.tensor_tensor(out=xl3, in0=xtf, in1=xh3, op=OP.subtract)
                nc.scalar.activation(out=xsq, in_=xtf, func=AF.Square)

                with tc.tile_pool(name="fpsum", bufs=1, space="PSUM") as fp:
                    sqp = fp.tile([1, N], F32, name="sqp")
                    for g in range(4):
                        nc.tensor.matmul(
                            sqp[:, 512 * g:512 * (g + 1)], lhsT=ones31,
                            rhs=xsq[:, 512 * g:512 * (g + 1)], start=True, stop=True,
                        )
                    nc.vector.tensor_scalar_mul(out=sneg, in0=sqp, scalar1=-0.5)
                    nc.vector.tensor_copy(out=shl, in_=sneg)
                    nc.vector.tensor_tensor(out=sll, in0=sneg, in1=shl, op=OP.subtract)
                    # engines only address partition starts {0,32,64,96}; DMA
                    # places single rows at arbitrary partitions.
                    # U rows: xh xh xl | -sqh/2 -sql/2 | 1 1
                    # V rows: xh xl xh |   1     1     | -sqh/2 -sql/2
                    nc.scalar.dma_start(out=U13[0:3, :], in_=xh3)
                    nc.scalar.dma_start(out=U13[3:6, :], in_=xh3)
                    nc.scalar.dma_start(out=U13[6:9, :], in_=xl3)
                    nc.scalar.dma_start(out=U13[9:10, :], in_=shl)
                    nc.scalar.dma_start(out=U13[10:11, :], in_=sll)
                    nc.scalar.dma_start(out=U13[11:13, :], in_=onesb2)
                    nc.sync.dma_start(out=V13[0:3, :], in_=xh3)
                    nc.sync.dma_start(out=V13[3:6, :], in_=xl3)
                    nc.sync.dma_start(out=V13[6:9, :], in_=xh3)
                    nc.sync.dma_start(out=V13[9:11, :], in_=onesb2)
                    nc.sync.dma_start(out=V13[11:12, :], in_=shl)
                    nc.sync.dma_start(out=V13[12:13, :], in_=sll)

                    # centroid = sum(x*m)/max(count,1); count = sum(m)
                    nc.vector.reduce_sum(out=mkr, in_=mk_s, axis=mybir.AxisListType.X)
                    cntp = fp.tile([1, 1], F32, name="cntp")
                    nc.tensor.matmul(cntp, lhsT=mkr, rhs=ones128, start=True, stop=True)
                    nc.vector.tensor_scalar_max(out=cnt_sb, in0=cntp, scalar1=1.0)
                    nc.vector.reciprocal(out=invc1, in_=cnt_sb)
                    invb = fp.tile([P, 1], F32, name="invb")
                    nc.tensor.matmul(invb, lhsT=ones1r, rhs=invc1, start=True, stop=True)
                    nc.vector.tensor_copy(out=invc_sb, in_=invb)
                    nc.vector.tensor_scalar_mul(out=msc, in0=mk_s, scalar1=invc_sb)
                    nc.vector.tensor_copy(out=msc_b, in_=msc)
                    centp = fp.tile([D, 1], F32, name="centp")
                    for r in range(R):
                        nc.tensor.matmul(
                            centp, lhsT=xb_s[:, D * r:D * (r + 1)], rhs=msc[:, r:r + 1],
                            start=(r == 0), stop=(r == R - 1),
                        )
                    nc.vector.tensor_copy(out=cent_sb, in_=centp)
                    nc.gpsimd.dma_start(out=cent_d[:, :], in_=cent_sb)
                    cent_ap = cent_d[:, :]
                    cbc = bass.AP(
                        tensor=cent_ap.tensor, offset=cent_ap.offset,
                        ap=[[0, 1], [0, R], [1, D]],
                    )
                    nc.gpsimd.dma_start(out=crow, in_=cbc)
                    c48p = fp.tile([P, R * D], F32, name="c48p")
                    nc.tensor.matmul(c48p, lhsT=ones1r, rhs=crow, start=True, stop=True)
                    nc.vector.tensor_tensor(out=h0, in0=xb_s, in1=c48p, op=OP.subtract)

                # ------------- phases 1+2 share the PSUM budget --------------
                with tc.tile_pool(name="spsum", bufs=2, space="PSUM") as sp, \
                     tc.tile_pool(name="apsum", bufs=2, space="PSUM") as apl, \
                     tc.tile_pool(name="bpsum", bufs=2, space="PSUM") as bpl:
                    # phase 1: E = exp(-dist), 16 row-blocks of [128, 2048]
                    for i in range(R):
                        for t in range(2):
                            ps = sp.tile([P, 1024], F32, name="ps", tag="ps")
                            for gg in range(2):
                                j0 = 1024 * t + 512 * gg
                                nc.tensor.matmul(
                                    ps[:, 512 * gg:512 * (gg + 1)],
                                    lhsT=U13[:, P * i:P * (i + 1)],
                                    rhs=V13[:, j0:j0 + 512], start=True, stop=True,
                                )
                            nc.scalar.activation(
                                out=E_all[:, N * i + 1024 * t: N * i + 1024 * (t + 1)],
                                in_=ps, func=AF.Exp, scale=2.0,
                            )

                    # phase 2: three message-passing layers
                    layers = [
                        (h0, D, w0_s, HID[0], h1),
                        (h1, HID[0], w1_s, HID[1], h2),
                        (h2, HID[1], w2_s, HID[2], h3),
                    ]
                    for li, (hin, d_in, w_s, d_out, hout) in enumerate(layers):
                        # (E @ h)^T accumulated over the 16 point-chunks
                        for g in range(4):
                            pa = apl.tile([P, 512], F32, name="pa", tag="pa")
                            for r in range(R):
                                nc.tensor.matmul(
                                    pa[:d_in, :], lhsT=hin[:, d_in * r:d_in * (r + 1)],
                                    rhs=E_all[:, N * r + 512 * g: N * r + 512 * (g + 1)],
                                    start=(r == 0), stop=(r == R - 1),
                                )
                            nc.vector.tensor_copy(
                                out=EhT[:d_in, 512 * g:512 * (g + 1)], in_=pa[:d_in, :]
                            )
                        # @W_aug, center, variance, rsqrt, scale, swish
                        for half in range(2):
                            for c in range(8 * half, 8 * half + 8):
                                pb = bpl.tile([P, d_out + 1], F32, name="pb", tag="pb")
                                ehc = EhT[:d_in, P * c:P * (c + 1)]
                                nc.tensor.matmul(
                                    pb, lhsT=ehc, rhs=w_s[:, 0:d_out + 1],
                                    start=True, stop=False,
                                )
                                nc.tensor.matmul(
                                    pb, lhsT=ehc,
                                    rhs=w_s[:, d_out + 1:2 * (d_out + 1)],
                                    start=False, stop=True,
                                )
                                ysl = y_all[:, d_out * c:d_out * (c + 1)]
                                # y0 = u - mean(u)   (psum col d_out holds -mean)
                                nc.vector.tensor_scalar(
                                    out=ysl, in0=pb[:, :d_out],
                                    scalar1=pb[:, d_out:d_out + 1], scalar2=None,
                                    op0=OP.add,
                                )
                                sqo = scr.tile([P, d_out], F32, name="sqo", tag="sqo")
                                nc.scalar.activation(
                                    out=sqo, in_=ysl, func=AF.Square,
                                    accum_out=varN[:, c:c + 1],
                                )
                            h8 = slice(8 * half, 8 * half + 8)
                            _emit_rsqrt(
                                nc, rstd[:, h8], varN[:, h8], rs_w, rs_t1, d_out
                            )
                            for c in range(8 * half, 8 * half + 8):
                                ysl = y_all[:, d_out * c:d_out * (c + 1)]
                                nc.vector.tensor_scalar_mul(
                                    out=ysl, in0=ysl, scalar1=rstd[:, c:c + 1]
                                )
                            yhalf = y_all[:, d_out * 8 * half:d_out * 8 * (half + 1)]
                            hhalf = hout[:, d_out * 8 * half:d_out * 8 * (half + 1)]
                            nc.scalar.activation(out=hhalf, in_=yhalf, func=AF.Silu)

                # ------------- phase 3: masked mean pool + readout -----------
                with tc.tile_pool(name="tpsum", bufs=1, space="PSUM") as tp:
                    gf0 = tp.tile([P, 1], F32, name="gf0")
                    gf1 = tp.tile([P, 1], F32, name="gf1")
                    for t, gft in enumerate((gf0, gf1)):
                        for c in range(R):
                            o = HID[2] * c + P * t
                            nc.tensor.matmul(
                                gft, lhsT=h3[:, o:o + P], rhs=msc_b[:, c:c + 1],
                                start=(c == 0), stop=(c == R - 1),
                            )
                    nc.vector.tensor_copy(out=gf_b[:, 0:1], in_=gf0)
                    nc.vector.tensor_copy(out=gf_b[:, 1:2], in_=gf1)
                    nc.vector.tensor_tensor(out=gfl_b[:, 0:1], in0=gf0,
                                            in1=gf_b[:, 0:1], op=OP.subtract)
                    nc.vector.tensor_tensor(out=gfl_b[:, 1:2], in0=gf1,
                                            in1=gf_b[:, 1:2], op=OP.subtract)
                    zps = tp.tile([P, 1], F32, name="zps")
                    # wz_s columns: [wzh half0 | wzh half1 | wzl half0 | wzl half1]
                    # z ~= Wzh.gfh + Wzl.gfh + Wzh.gfl   (drop Wzl.gfl)
                    zmm = [(0, gf_b, 0), (1, gf_b, 1), (2, gf_b, 0), (3, gf_b, 1),
                           (0, gfl_b, 0), (1, gfl_b, 1)]
                    for k, (wcol, gsrc, gcol) in enumerate(zmm):
                        nc.tensor.matmul(
                            zps, lhsT=wz_s[:, LAT * wcol:LAT * (wcol + 1)],
                            rhs=gsrc[:, gcol:gcol + 1],
                            start=(k == 0), stop=(k == len(zmm) - 1),
                        )
                    nc.vector.scalar_tensor_tensor(
                        out=z_sb, in0=zps, scalar=1.0, in1=bz_s,
                        op0=OP.mult, op1=OP.add,
                    )
                    nc.sync.dma_start(out=zout[:, :], in_=z_sb)
    return nc


_NC_CACHE = None


def _get_nc():
    global _NC_CACHE
    if _NC_CACHE is None:
        _apply_tile_patch()
        nc = _build()
        nc.finalize()   # Bacc.compile(): wait legalization + register alloc
        _NC_CACHE = nc
    return _NC_CACHE


def _host_prep(inputs):
    x = np.asarray(inputs["x"], np.float32)
    mask = np.asarray(inputs["mask"], np.float32)
    W = [np.asarray(inputs[f"W{i}"], np.float32) for i in range(3)]
    Wz = np.asarray(inputs["Wz"], np.float32)
    bz = np.asarray(inputs["bz"], np.float32)

    def hilo(a):
        hi = a.astype(ml_dtypes.bfloat16)
        lo = (a - hi.astype(np.float32)).astype(ml_dtypes.bfloat16)
        return hi, lo

    waug = []
    for i in range(3):
        a = np.concatenate([W[i], -W[i].mean(axis=1, keepdims=True)], axis=1)
        hi, lo = hilo(a)
        waug.append(np.ascontiguousarray(np.concatenate([hi, lo], axis=1)))
    wzflat = np.concatenate([Wz[:P, :], Wz[P:, :]], axis=1)
    wzh, wzl = hilo(wzflat)
    wz2 = np.ascontiguousarray(np.concatenate([wzh, wzl], axis=1))
    bzr = np.ascontiguousarray(bz.reshape(P, 1))

    in_maps = []
    for bi in range(B):
        in_maps.append({
            "xt": np.ascontiguousarray(
                x[bi].reshape(P, R, D).transpose(2, 1, 0).reshape(D, N)
            ),
            "xb": np.ascontiguousarray(x[bi].reshape(P, R * D)),
            "mk": np.ascontiguousarray(mask[bi].reshape(P, R)),
            "w0a": waug[0], "w1a": waug[1], "w2a": waug[2],
            "wz2": wz2, "bzt": bzr,
        })
    return in_maps


def kernel(**inputs):
    for i in range(3):
        if (np.any(np.asarray(inputs[f"b{i}"])) or
                np.any(np.asarray(inputs[f"be{i}"])) or
                np.any(np.asarray(inputs[f"g{i}"]) != 1.0)):
            raise NotImplementedError(
                "kernel specialized for zero LN/layer biases and unit gains"
            )
    in_maps = _host_prep(inputs)
    nc = _get_nc()
    res = run_bass_kernel_spmd(nc, in_maps, core_ids=list(range(B)))
    return np.stack([res.results[i]["z"][:, 0] for i in range(B)]).astype(np.float32)


# revision 18
# speedup vs baseline: 1.1991x; 1.0347x over previous
"""Trainium2 Bass kernel for nn_EquivariantDeepSetsEncoder.

Strategy: data-parallel over batch (B=8) across 8 NeuronCores; one batch per
core. Per core the full 2048x2048 attention matrix E = exp(-pairwise_dist)
stays resident in SBUF (8 MB in bf16) and is reused by all three
message-passing layers, so HBM traffic is just the tiny inputs/outputs.

Key algebraic simplifications (exact, not approximations):
  * pairwise distances depend only on coordinate differences, so the centroid
    subtraction cancels inside the attention; E is built from raw x.
  * -dist(i,j) = x_i.x_j - |x_i|^2/2 - |x_j|^2/2 (then exp with scale=2),
    which a single augmented matmul computes.
  * softmax row-normalization (1/rowsum) cancels through LayerNorm's scale
    invariance because the pre-LN bias b_i is zero, so it is never computed.
  * LayerNorm mean comes for free as an extra (negated, pre-averaged) column
    of the layer weight matrix.

Precision: fp32 matmuls stream at 4 cycles/column on the PE, bf16 at 1. The
attention logits need ~fp32 accuracy, so they use a hi/lo split-precision
bf16 matmul (x = xh + xl, keeping the xh*xh + xh*xl + xl*xh terms, same for
the squared norms; K grows 5 -> 13 but moving cycles drop 4x). E, h and W
are plain bf16 (their quantization error averages out over 2048 points);
LayerNorm statistics and activations are computed in fp32.
"""

import math
import os

import numpy as np
import ml_dtypes

import concourse.bass as bass
import concourse.bacc as bacc
import concourse.mybir as mybir
import concourse.tile as tile
from concourse.bass_utils import run_bass_kernel_spmd
from concourse.vector_clock import ScopedClock

F32 = mybir.dt.float32
BF16 = mybir.dt.bfloat16
U32 = mybir.dt.uint32
AF = mybir.ActivationFunctionType
OP = mybir.AluOpType

B, N, D = 8, 2048, 3
P, R = 128, 16          # N = P * R; point (p, r) = original index 16*p + r
HID = (64, 128, 256)
LAT = 128
EPS = 1e-6
RSQRT_MAGIC = 0x5F3759DF


# ---------------------------------------------------------------------------
# Workaround for a walrus codegen limit in this toolchain: a NO_STRUCT
# instruction (Drain) can carry at most one sync-wait command. Tile's exit
# path attaches the full global-clock wait set to a single drain; split the
# waits across several drains instead. (Bacc's later legalization handles
# the rest of the instructions.)
def _split_drain_and_barrier(self, tick_clock, wait_clock):
    nc = self.nc
    drain_inst = nc.sync.drain()
    wait_clock.add_sem_waits(
        drain_inst.ins, ScopedClock({None: tick_clock.global_clock})
    )
    si = drain_inst.ins.sync_info
    waits = list(si.on_wait) if si is not None else []
    if len(waits) > 1:
        si.on_wait = [waits[0]]
        for w in waits[1:]:
            d2 = nc.sync.drain()
            if d2.ins.sync_info is not None:
                d2.ins.sync_info.on_wait = [w]
            else:
                d2.ins.sync_info = mybir.SyncInfo(on_wait=[w], on_update=[])
    nc.all_engine_barrier()
    assert self.sems is not None
    popped = nc._tile_sem_poison_stack.pop()
    assert popped is self._sem_poison
    nc.clear_and_free_semaphores(list(self.sems.allocated().values()))
    nc.all_engine_barrier()


def _apply_tile_patch():
    if os.environ.get("NO_DRAIN_PATCH", "0") == "1":
        return
    tile.TileContext._drain_and_barrier = _split_drain_and_barrier


# ---------------------------------------------------------------------------
def _emit_rsqrt(nc, out_ap, var_ap, w_t, t1_t, d_out):
    """out = sqrt(d_out / (var_ap + d_out*EPS)) == 1/sqrt(var + EPS), where
    var_ap holds sum-of-squares (d_out * var). Fast-inverse-sqrt seed plus
    three Newton iterations, fp32-accurate; DVE only (no activation table)."""
    nc.vector.tensor_single_scalar(out=w_t, in_=var_ap, scalar=d_out * EPS, op=OP.add)
    w_u = w_t.bitcast(U32)
    t1_u = t1_t.bitcast(U32)
    # seed bits = MAGIC - (w_bits >> 1). The DVE ALU is fp32 for +/-, so the
    # subtraction happens in float on the integer VALUES (result stays in
    # [5e8, 1.6e9], no wrap) and the uint32 output cast restores the bits;
    # the ~1e-5 relative bit noise is far below the seed's 3.4% error.
    nc.vector.tensor_scalar(
        out=t1_u, in0=w_u, scalar1=1, scalar2=None, op0=OP.logical_shift_right,
    )
    r_t = out_ap
    r_u = r_t.bitcast(U32)
    nc.vector.tensor_scalar(
        out=r_u, in0=t1_u, scalar1=-1.0, scalar2=float(RSQRT_MAGIC),
        op0=OP.mult, op1=OP.add,
    )
    sqd = math.sqrt(float(d_out))
    for it in range(2):
        nc.vector.tensor_tensor(out=t1_t, in0=r_t, in1=r_t, op=OP.mult)
        nc.vector.tensor_tensor(out=t1_t, in0=t1_t, in1=w_t, op=OP.mult)
        nc.vector.tensor_scalar(
            out=t1_t, in0=t1_t, scalar1=-0.5, scalar2=1.5, op0=OP.mult, op1=OP.add
        )
        if it < 1:
            nc.vector.tensor_tensor(out=r_t, in0=r_t, in1=t1_t, op=OP.mult)
        else:
            # fold the sqrt(d_out) factor into the final Newton multiply
            nc.vector.scalar_tensor_tensor(
                out=r_t, in0=r_t, scalar=sqd, in1=t1_t, op0=OP.mult, op1=OP.mult
            )


def _build(reps=1):
    nc = bacc.Bacc()
    xt = nc.dram_tensor("xt", [D, N], F32, kind="ExternalInput")
    xb = nc.dram_tensor("xb", [P, R * D], F32, kind="ExternalInput")
    mk = nc.dram_tensor("mk", [P, R], F32, kind="ExternalInput")
    w0a = nc.dram_tensor("w0a", [D, 2 * (HID[0] + 1)], BF16, kind="ExternalInput")
    w1a = nc.dram_tensor("w1a", [HID[0], 2 * (HID[1] + 1)], BF16, kind="ExternalInput")
    w2a = nc.dram_tensor("w2a", [HID[1], 2 * (HID[2] + 1)], BF16, kind="ExternalInput")
    wz2 = nc.dram_tensor("wz2", [P, 4 * LAT], BF16, kind="ExternalInput")
    bzt = nc.dram_tensor("bzt", [P, 1], F32, kind="ExternalInput")
    zout = nc.dram_tensor("z", [P, 1], F32, kind="ExternalOutput")
    cent_d = nc.dram_tensor("cent_scratch", [1, D], F32, kind="Internal")

    with tile.TileContext(nc) as tc:
        with tc.tile_pool(name="persist", bufs=1) as pp, \
             tc.tile_pool(name="scr", bufs=2) as scr:
            E_all = pp.tile([P, R * N], BF16, name="E_all")
            U13 = pp.tile([36, N], BF16, name="U13")
            V13 = pp.tile([36, N], BF16, name="V13")
            xtf = pp.tile([D, N], F32, name="xtf")
            xh3 = pp.tile([D, N], BF16, name="xh3")
            xl3 = pp.tile([D, N], BF16, name="xl3")
            xsq = pp.tile([D, N], F32, name="xsq")
            sneg = pp.tile([1, N], F32, name="sneg")
            shl = pp.tile([1, N], BF16, name="shl")
            sll = pp.tile([1, N], BF16, name="sll")
            onesb2 = pp.tile([2, N], BF16, name="onesb2")
            h0 = pp.tile([P, R * D], BF16, name="h0")
            h1 = pp.tile([P, R * HID[0]], BF16, name="h1")
            h2 = pp.tile([P, R * HID[1]], BF16, name="h2")
            h3 = pp.tile([P, R * HID[2]], BF16, name="h3")
            y_all = pp.tile([P, R * HID[2]], F32, name="y_all")
            EhT = pp.tile([P, N], BF16, name="EhT")
            xb_s = pp.tile([P, R * D], F32, name="xb_s")
            mk_s = pp.tile([P, R], F32, name="mk_s")
            msc = pp.tile([P, R], F32, name="msc")
            msc_b = pp.tile([P, R], BF16, name="msc_b")
            crow = pp.tile([1, R * D], F32, name="crow")
            w0_s = pp.tile([D, 2 * (HID[0] + 1)], BF16, name="w0_s")
            w1_s = pp.tile([HID[0], 2 * (HID[1] + 1)], BF16, name="w1_s")
            w2_s = pp.tile([HID[1], 2 * (HID[2] + 1)], BF16, name="w2_s")
            wz_s = pp.tile([P, 4 * LAT], BF16, name="wz_s")
            gfl_b = pp.tile([P, 2], BF16, name="gfl_b")
            bz_s = pp.tile([P, 1], F32, name="bz_s")
            ones31 = pp.tile([D, 1], F32, name="ones31")
            ones128 = pp.tile([P, 1], F32, name="ones128")
            ones1r = pp.tile([1, P], F32, name="ones1r")
            mkr = pp.tile([P, 1], F32, name="mkr")
            cnt_sb = pp.tile([1, 1], F32, name="cnt_sb")
            invc1 = pp.tile([1, 1], F32, name="invc1")
            invc_sb = pp.tile([P, 1], F32, name="invc_sb")
            cent_sb = pp.tile([D, 1], F32, name="cent_sb")
            varN = pp.tile([P, R], F32, name="varN")
            rstd = pp.tile([P, R], F32, name="rstd")
            rs_w = pp.tile([P, 8], F32, name="rs_w")
            rs_t1 = pp.tile([P, 8], F32, name="rs_t1")
            gf_b = pp.tile([P, 2], BF16, name="gf_b")
            z_sb = pp.tile([P, 1], F32, name="z_sb")

            for _rep in range(reps):
                # ------------- front: loads, hi/lo U/V build, centroid -------
                nc.sync.dma_start(out=xtf, in_=xt[:, :])
                nc.scalar.dma_start(out=xb_s, in_=xb[:, :])
                nc.scalar.dma_start(out=mk_s, in_=mk[:, :])
                nc.gpsimd.dma_start(out=w0_s, in_=w0a[:, :])
                nc.gpsimd.dma_start(out=w1_s, in_=w1a[:, :])
                nc.gpsimd.dma_start(out=w2_s, in_=w2a[:, :])
                nc.gpsimd.dma_start(out=wz_s, in_=wz2[:, :])
                nc.gpsimd.dma_start(out=bz_s, in_=bzt[:, :])
                nc.vector.memset(onesb2, 1.0)
                # rows 9..31 stay zero and contribute nothing to the K=36 matmul
                nc.vector.memset(U13, 0.0)
                nc.vector.memset(V13, 0.0)
                nc.gpsimd.memset(ones31, 1.0)
                nc.gpsimd.memset(ones128, 1.0)
                nc.gpsimd.memset(ones1r, 1.0)
                # hi/lo split of the coordinates
                nc.vector.tensor_copy(out=xh3, in_=xtf)
                nc.vector.tensor_tensor(out=xl3, in0=xtf, in1=xh3, op=OP.subtract)
                nc.scalar.activation(out=xsq, in_=xtf, func=AF.Square)

                with tc.tile_pool(name="fpsum", bufs=1, space="PSUM") as fp:
                    sqp = fp.tile([1, N], F32, name="sqp")
                    for g in range(4):
                        nc.tensor.matmul(
                            sqp[:, 512 * g:512 * (g + 1)], lhsT=ones31,
                            rhs=xsq[:, 512 * g:512 * (g + 1)], start=True, stop=True,
                        )
                    nc.vector.tensor_scalar_mul(out=sneg, in0=sqp, scalar1=-0.5)
                    nc.vector.tensor_copy(out=shl, in_=sneg)
                    nc.vector.tensor_tensor(out=sll, in0=sneg, in1=shl, op=OP.subtract)
                    # engines only address partition starts {0,32,64,96}; DMA
                    # places single rows at arbitrary partitions.
                    # U rows: xh xh xl | -sqh/2 -sql/2 | 1 1
                    # V rows: xh xl xh |   1     1     | -sqh/2 -sql/2
                    nc.scalar.dma_start(out=U13[0:3, :], in_=xh3)
                    nc.scalar.dma_start(out=U13[3:6, :], in_=xh3)
                    nc.scalar.dma_start(out=U13[6:9, :], in_=xl3)
                    nc.scalar.dma_start(out=U13[32:33, :], in_=shl)
                    nc.scalar.dma_start(out=U13[33:34, :], in_=sll)
                    nc.scalar.dma_start(out=U13[34:36, :], in_=onesb2)
                    nc.sync.dma_start(out=V13[0:3, :], in_=xh3)
                    nc.sync.dma_start(out=V13[3:6, :], in_=xl3)
                    nc.sync.dma_start(out=V13[6:9, :], in_=xh3)
                    nc.sync.dma_start(out=V13[32:34, :], in_=onesb2)
                    nc.sync.dma_start(out=V13[34:35, :], in_=shl)
                    nc.sync.dma_start(out=V13[35:36, :], in_=sll)

                    # centroid = sum(x*m)/max(count,1); count = sum(m)
                    nc.vector.reduce_sum(out=mkr, in_=mk_s, axis=mybir.AxisListType.X)
                    cntp = fp.tile([1, 1], F32, name="cntp")
                    nc.tensor.matmul(cntp, lhsT=mkr, rhs=ones128, start=True, stop=True)
                    nc.vector.tensor_scalar_max(out=cnt_sb, in0=cntp, scalar1=1.0)
                    nc.vector.reciprocal(out=invc1, in_=cnt_sb)
                    invb = fp.tile([P, 1], F32, name="invb")
                    nc.tensor.matmul(invb, lhsT=ones1r, rhs=invc1, start=True, stop=True)
                    nc.vector.tensor_copy(out=invc_sb, in_=invb)
                    nc.vector.tensor_scalar_mul(out=msc, in0=mk_s, scalar1=invc_sb)
                    nc.vector.tensor_copy(out=msc_b, in_=msc)
                    centp = fp.tile([D, 1], F32, name="centp")
                    for r in range(R):
                        nc.tensor.matmul(
                            centp, lhsT=xb_s[:, D * r:D * (r + 1)], rhs=msc[:, r:r + 1],
                            start=(r == 0), stop=(r == R - 1),
                        )
                    nc.vector.tensor_copy(out=cent_sb, in_=centp)
                    nc.gpsimd.dma_start(out=cent_d[:, :], in_=cent_sb)
                    cent_ap = cent_d[:, :]
                    cbc = bass.AP(
                        tensor=cent_ap.tensor, offset=cent_ap.offset,
                        ap=[[0, 1], [0, R], [1, D]],
                    )
                    nc.gpsimd.dma_start(out=crow, in_=cbc)
                    c48p = fp.tile([P, R * D], F32, name="c48p")
                    nc.tensor.matmul(c48p, lhsT=ones1r, rhs=crow, start=True, stop=True)
                    nc.vector.tensor_tensor(out=h0, in0=xb_s, in1=c48p, op=OP.subtract)

                # ------------- phases 1+2 share the PSUM budget --------------
                with tc.tile_pool(name="spsum", bufs=2, space="PSUM") as sp, \
                     tc.tile_pool(name="apsum", bufs=2, space="PSUM") as apl, \
                     tc.tile_pool(name="bpsum", bufs=2, space="PSUM") as bpl:
                    # phase 1: E = exp(-dist), 16 row-blocks of [128, 2048]
                    for i in range(R):
                        for t in range(2):
                            ps = sp.tile([P, 1024], F32, name="ps", tag="ps")
                            for gg in range(2):
                                j0 = 1024 * t + 512 * gg
                                nc.tensor.matmul(
                                    ps[:, 512 * gg:512 * (gg + 1)],
                                    lhsT=U13[0:36, P * i:P * (i + 1)],
                                    rhs=V13[0:36, j0:j0 + 512],
                                    start=True, stop=True,
                                )
                            nc.scalar.activation(
                                out=E_all[:, N * i + 1024 * t: N * i + 1024 * (t + 1)],
                                in_=ps, func=AF.Exp, scale=2.0,
                            )

                    # phase 2: three message-passing layers
                    layers = [
                        (h0, D, w0_s, HID[0], h1),
                        (h1, HID[0], w1_s, HID[1], h2),
                        (h2, HID[1], w2_s, HID[2], h3),
                    ]
                    for li, (hin, d_in, w_s, d_out, hout) in enumerate(layers):
                        # (E @ h)^T accumulated over the 16 point-chunks
                        for g in range(4):
                            pa = apl.tile([P, 512], F32, name="pa", tag="pa")
                            for r in range(R):
                                nc.tensor.matmul(
                                    pa[:d_in, :], lhsT=hin[:, d_in * r:d_in * (r + 1)],
                                    rhs=E_all[:, N * r + 512 * g: N * r + 512 * (g + 1)],
                                    start=(r == 0), stop=(r == R - 1),
                                )
                            nc.vector.tensor_copy(
                                out=EhT[:d_in, 512 * g:512 * (g + 1)], in_=pa[:d_in, :]
                            )
                        # @W_aug, center, variance, rsqrt, scale, swish
                        for half in range(2):
                            for c in range(8 * half, 8 * half + 8):
                                pb = bpl.tile([P, d_out + 1], F32, name="pb", tag="pb")
                                ehc = EhT[:d_in, P * c:P * (c + 1)]
                                nc.tensor.matmul(
                                    pb, lhsT=ehc, rhs=w_s[:, 0:d_out + 1],
                                    start=True, stop=False,
                                )
                                nc.tensor.matmul(
                                    pb, lhsT=ehc,
                                    rhs=w_s[:, d_out + 1:2 * (d_out + 1)],
                                    start=False, stop=True,
                                )
                                ysl = y_all[:, d_out * c:d_out * (c + 1)]
                                # y0 = u - mean(u)   (psum col d_out holds -mean)
                                nc.vector.tensor_scalar(
                                    out=ysl, in0=pb[:, :d_out],
                                    scalar1=pb[:, d_out:d_out + 1], scalar2=None,
                                    op0=OP.add,
                                )
                                sqo = scr.tile([P, d_out], F32, name="sqo", tag="sqo")
                                nc.scalar.activation(
                                    out=sqo, in_=ysl, func=AF.Square,
                                    accum_out=varN[:, c:c + 1],
                                )
                            h8 = slice(8 * half, 8 * half + 8)
                            _emit_rsqrt(
                                nc, rstd[:, h8], varN[:, h8], rs_w, rs_t1, d_out
                            )
                            for c in range(8 * half, 8 * half + 8):
                                ysl = y_all[:, d_out * c:d_out * (c + 1)]
                                nc.vector.tensor_scalar_mul(
                                    out=ysl, in0=ysl, scalar1=rstd[:, c:c + 1]
                                )
                            yhalf = y_all[:, d_out * 8 * half:d_out * 8 * (half + 1)]
                            hhalf = hout[:, d_out * 8 * half:d_out * 8 * (half + 1)]
                            nc.scalar.activation(out=hhalf, in_=yhalf, func=AF.Silu)

                # ------------- phase 3: masked mean pool + readout -----------
                with tc.tile_pool(name="tpsum", bufs=1, space="PSUM") as tp:
                    gf0 = tp.tile([P, 1], F32, name="gf0")
                    gf1 = tp.tile([P, 1], F32, name="gf1")
                    for t, gft in enumerate((gf0, gf1)):
                        for c in range(R):
                            o = HID[2] * c + P * t
                            nc.tensor.matmul(
                                gft, lhsT=h3[:, o:o + P], rhs=msc_b[:, c:c + 1],
                                start=(c == 0), stop=(c == R - 1),
                            )
                    nc.vector.tensor_copy(out=gf_b[:, 0:1], in_=gf0)
                    nc.vector.tensor_copy(out=gf_b[:, 1:2], in_=gf1)
                    nc.vector.tensor_tensor(out=gfl_b[:, 0:1], in0=gf0,
                                            in1=gf_b[:, 0:1], op=OP.subtract)
                    nc.vector.tensor_tensor(out=gfl_b[:, 1:2], in0=gf1,
                                            in1=gf_b[:, 1:2], op=OP.subtract)
                    zps = tp.tile([P, 1], F32, name="zps")
                    # wz_s columns: [wzh half0 | wzh half1 | wzl half0 | wzl half1]
                    # z ~= Wzh.gfh + Wzl.gfh + Wzh.gfl   (drop Wzl.gfl)
                    zmm = [(0, gf_b, 0), (1, gf_b, 1), (2, gf_b, 0), (3, gf_b, 1),
                           (0, gfl_b, 0), (1, gfl_b, 1)]
                    for k, (wcol, gsrc, gcol) in enumerate(zmm):
                        nc.tensor.matmul(
                            zps, lhsT=wz_s[:, LAT * wcol:LAT * (wcol + 1)],
                            rhs=gsrc[:, gcol:gcol + 1],
                            start=(k == 0), stop=(k == len(zmm) - 1),
                        )
                    nc.vector.scalar_tensor_tensor(
                        out=z_sb, in0=zps, scalar=1.0, in1=bz_s,
                        op0=OP.mult, op1=OP.add,
                    )
                    nc.sync.dma_start(out=zout[:, :], in_=z_sb)
    return nc


_NC_CACHE = None


def _get_nc():
    global _NC_CACHE
    if _NC_CACHE is None:
        _apply_tile_patch()
        nc = _build()
        nc.finalize()   # Bacc.compile(): wait legalization + register alloc
        _NC_CACHE = nc
    return _NC_CACHE


def _host_prep(inputs):
    x = np.asarray(inputs["x"], np.float32)
    mask = np.asarray(inputs["mask"], np.float32)
    W = [np.asarray(inputs[f"W{i}"], np.float32) for i in range(3)]
    Wz = np.asarray(inputs["Wz"], np.float32)
    bz = np.asarray(inputs["bz"], np.float32)

    def hilo(a):
        hi = a.astype(ml_dtypes.bfloat16)
        lo = (a - hi.astype(np.float32)).astype(ml_dtypes.bfloat16)
        return hi, lo

    waug = []
    for i in range(3):
        a = np.concatenate([W[i], -W[i].mean(axis=1, keepdims=True)], axis=1)
        hi, lo = hilo(a)
        waug.append(np.ascontiguousarray(np.concatenate([hi, lo], axis=1)))
    wzflat = np.concatenate([Wz[:P, :], Wz[P:, :]], axis=1)
    wzh, wzl = hilo(wzflat)
    wz2 = np.ascontiguousarray(np.concatenate([wzh, wzl], axis=1))
    bzr = np.ascontiguousarray(bz.reshape(P, 1))

    in_maps = []
    for bi in range(B):
        in_maps.append({
            "xt": np.ascontiguousarray(
                x[bi].reshape(P, R, D).transpose(2, 1, 0).reshape(D, N)
            ),
            "xb": np.ascontiguousarray(x[bi].reshape(P, R * D)),
            "mk": np.ascontiguousarray(mask[bi].reshape(P, R)),
            "w0a": waug[0], "w1a": waug[1], "w2a": waug[2],
            "wz2": wz2, "bzt": bzr,
        })
    return in_maps


def kernel(**inputs):
    for i in range(3):
        if (np.any(np.asarray(inputs[f"b{i}"])) or
                np.any(np.asarray(inputs[f"be{i}"])) or
                np.any(np.asarray(inputs[f"g{i}"]) != 1.0)):
            raise NotImplementedError(
                "kernel specialized for zero LN/layer biases and unit gains"
            )
    in_maps = _host_prep(inputs)
    nc = _get_nc()
    res = run_bass_kernel_spmd(nc, in_maps, core_ids=list(range(B)))
    return np.stack([res.results[i]["z"][:, 0] for i in range(B)]).astype(np.float32)


# revision 19
# speedup vs baseline: 1.3393x; 1.1169x over previous
"""Trainium2 Bass kernel for nn_EquivariantDeepSetsEncoder.

Strategy: data-parallel over batch (B=8) across 8 NeuronCores; one batch per
core. Per core the full 2048x2048 attention matrix E = exp(-pairwise_dist)
stays resident in SBUF (8 MB in bf16) and is reused by all three
message-passing layers, so HBM traffic is just the tiny inputs/outputs.

Key algebraic simplifications (exact, not approximations):
  * pairwise distances depend only on coordinate differences, so the centroid
    subtraction cancels inside the attention; E is built from raw x.
  * -dist(i,j) = x_i.x_j - |x_i|^2/2 - |x_j|^2/2 (then exp with scale=2),
    which a single augmented matmul computes.
  * softmax row-normalization (1/rowsum) cancels through LayerNorm's scale
    invariance because the pre-LN bias b_i is zero, so it is never computed.
  * LayerNorm mean comes for free as an extra (negated, pre-averaged) column
    of the layer weight matrix.

Precision: fp32 matmuls stream at 4 cycles/column on the PE, bf16 at 1. The
attention logits need ~fp32 accuracy, so they use a hi/lo split-precision
bf16 matmul (x = xh + xl, keeping the xh*xh + xh*xl + xl*xh terms, same for
the squared norms; K grows 5 -> 13 but moving cycles drop 4x). E, h and W
are plain bf16 (their quantization error averages out over 2048 points);
LayerNorm statistics and activations are computed in fp32.
"""

import math
import os

import numpy as np
import ml_dtypes

import concourse.bass as bass
import concourse.bacc as bacc
import concourse.mybir as mybir
import concourse.tile as tile
from concourse.bass_utils import run_bass_kernel_spmd
from concourse.vector_clock import ScopedClock

F32 = mybir.dt.float32
BF16 = mybir.dt.bfloat16
U32 = mybir.dt.uint32
AF = mybir.ActivationFunctionType
OP = mybir.AluOpType

B, N, D = 8, 2048, 3
P, R = 128, 16          # N = P * R; point (p, r) = original index 16*p + r
HID = (64, 128, 256)
LAT = 128
EPS = 1e-6
RSQRT_MAGIC = 0x5F3759DF


# ---------------------------------------------------------------------------
# Workaround for a walrus codegen limit in this toolchain: a NO_STRUCT
# instruction (Drain) can carry at most one sync-wait command. Tile's exit
# path attaches the full global-clock wait set to a single drain; split the
# waits across several drains instead. (Bacc's later legalization handles
# the rest of the instructions.)
def _split_drain_and_barrier(self, tick_clock, wait_clock):
    nc = self.nc
    drain_inst = nc.sync.drain()
    wait_clock.add_sem_waits(
        drain_inst.ins, ScopedClock({None: tick_clock.global_clock})
    )
    si = drain_inst.ins.sync_info
    waits = list(si.on_wait) if si is not None else []
    if len(waits) > 1:
        si.on_wait = [waits[0]]
        for w in waits[1:]:
            d2 = nc.sync.drain()
            if d2.ins.sync_info is not None:
                d2.ins.sync_info.on_wait = [w]
            else:
                d2.ins.sync_info = mybir.SyncInfo(on_wait=[w], on_update=[])
    nc.all_engine_barrier()
    assert self.sems is not None
    popped = nc._tile_sem_poison_stack.pop()
    assert popped is self._sem_poison
    nc.clear_and_free_semaphores(list(self.sems.allocated().values()))
    nc.all_engine_barrier()


def _apply_tile_patch():
    if os.environ.get("NO_DRAIN_PATCH", "0") == "1":
        return
    tile.TileContext._drain_and_barrier = _split_drain_and_barrier


# ---------------------------------------------------------------------------
def _emit_rsqrt(nc, out_ap, var_ap, w_t, t1_t, d_out):
    """out = sqrt(d_out / (var_ap + d_out*EPS)) == 1/sqrt(var + EPS), where
    var_ap holds sum-of-squares (d_out * var). Fast-inverse-sqrt seed plus
    three Newton iterations, fp32-accurate; DVE only (no activation table)."""
    nc.vector.tensor_single_scalar(out=w_t, in_=var_ap, scalar=d_out * EPS, op=OP.add)
    w_u = w_t.bitcast(U32)
    t1_u = t1_t.bitcast(U32)
    # seed bits = MAGIC - (w_bits >> 1). The DVE ALU is fp32 for +/-, so the
    # subtraction happens in float on the integer VALUES (result stays in
    # [5e8, 1.6e9], no wrap) and the uint32 output cast restores the bits;
    # the ~1e-5 relative bit noise is far below the seed's 3.4% error.
    nc.vector.tensor_scalar(
        out=t1_u, in0=w_u, scalar1=1, scalar2=None, op0=OP.logical_shift_right,
    )
    r_t = out_ap
    r_u = r_t.bitcast(U32)
    nc.vector.tensor_scalar(
        out=r_u, in0=t1_u, scalar1=-1.0, scalar2=float(RSQRT_MAGIC),
        op0=OP.mult, op1=OP.add,
    )
    sqd = math.sqrt(float(d_out))
    for it in range(2):
        nc.vector.tensor_tensor(out=t1_t, in0=r_t, in1=r_t, op=OP.mult)
        nc.vector.tensor_tensor(out=t1_t, in0=t1_t, in1=w_t, op=OP.mult)
        nc.vector.tensor_scalar(
            out=t1_t, in0=t1_t, scalar1=-0.5, scalar2=1.5, op0=OP.mult, op1=OP.add
        )
        if it < 1:
            nc.vector.tensor_tensor(out=r_t, in0=r_t, in1=t1_t, op=OP.mult)
        else:
            # fold the sqrt(d_out) factor into the final Newton multiply
            nc.vector.scalar_tensor_tensor(
                out=r_t, in0=r_t, scalar=sqd, in1=t1_t, op0=OP.mult, op1=OP.mult
            )


def _build(reps=1):
    nc = bacc.Bacc()
    xt = nc.dram_tensor("xt", [D, N], F32, kind="ExternalInput")
    xb = nc.dram_tensor("xb", [P, R * D], F32, kind="ExternalInput")
    mk = nc.dram_tensor("mk", [P, R], F32, kind="ExternalInput")
    w0a = nc.dram_tensor("w0a", [D, 2 * (HID[0] + 1)], BF16, kind="ExternalInput")
    w1a = nc.dram_tensor("w1a", [HID[0], 2 * (HID[1] + 1)], BF16, kind="ExternalInput")
    w2a = nc.dram_tensor("w2a", [HID[1], 2 * (HID[2] + 1)], BF16, kind="ExternalInput")
    wz2 = nc.dram_tensor("wz2", [P, 4 * LAT], BF16, kind="ExternalInput")
    bzt = nc.dram_tensor("bzt", [P, 1], F32, kind="ExternalInput")
    zout = nc.dram_tensor("z", [P, 1], F32, kind="ExternalOutput")
    cent_d = nc.dram_tensor("cent_scratch", [1, D], F32, kind="Internal")

    with tile.TileContext(nc) as tc:
        with tc.tile_pool(name="persist", bufs=1) as pp, \
             tc.tile_pool(name="scr", bufs=2) as scr:
            E_all = pp.tile([P, R * N], BF16, name="E_all")
            U13 = pp.tile([36, N], BF16, name="U13")
            V13 = pp.tile([36, N], BF16, name="V13")
            xtf = pp.tile([D, N], F32, name="xtf")
            xh3 = pp.tile([D, N], BF16, name="xh3")
            xl3 = pp.tile([D, N], BF16, name="xl3")
            xsq = pp.tile([D, N], F32, name="xsq")
            sneg = pp.tile([1, N], F32, name="sneg")
            shl = pp.tile([1, N], BF16, name="shl")
            sll = pp.tile([1, N], BF16, name="sll")
            onesb2 = pp.tile([2, N], BF16, name="onesb2")
            h0 = pp.tile([P, R * D], BF16, name="h0")
            h1 = pp.tile([P, R * HID[0]], BF16, name="h1")
            h2 = pp.tile([P, R * HID[1]], BF16, name="h2")
            h3 = pp.tile([P, R * HID[2]], BF16, name="h3")
            y_all = pp.tile([P, R * HID[2]], F32, name="y_all")
            EhT = pp.tile([P, N], BF16, name="EhT")
            xb_s = pp.tile([P, R * D], F32, name="xb_s")
            mk_s = pp.tile([P, R], F32, name="mk_s")
            msc = pp.tile([P, R], F32, name="msc")
            msc_b = pp.tile([P, R], BF16, name="msc_b")
            crow = pp.tile([1, R * D], F32, name="crow")
            w0_s = pp.tile([D, 2 * (HID[0] + 1)], BF16, name="w0_s")
            w1_s = pp.tile([HID[0], 2 * (HID[1] + 1)], BF16, name="w1_s")
            w2_s = pp.tile([HID[1], 2 * (HID[2] + 1)], BF16, name="w2_s")
            wz_s = pp.tile([P, 4 * LAT], BF16, name="wz_s")
            gfl_b = pp.tile([P, 2], BF16, name="gfl_b")
            bz_s = pp.tile([P, 1], F32, name="bz_s")
            ones31 = pp.tile([D, 1], F32, name="ones31")
            ones128 = pp.tile([P, 1], F32, name="ones128")
            ones1r = pp.tile([1, P], F32, name="ones1r")
            mkr = pp.tile([P, 1], F32, name="mkr")
            cnt_sb = pp.tile([1, 1], F32, name="cnt_sb")
            invc1 = pp.tile([1, 1], F32, name="invc1")
            invc_sb = pp.tile([P, 1], F32, name="invc_sb")
            cent_sb = pp.tile([D, 1], F32, name="cent_sb")
            varN = pp.tile([P, R], F32, name="varN")
            rstd = pp.tile([P, R], F32, name="rstd")
            rs_w = pp.tile([P, 8], F32, name="rs_w")
            rs_t1 = pp.tile([P, 8], F32, name="rs_t1")
            gf_b = pp.tile([P, 2], BF16, name="gf_b")
            z_sb = pp.tile([P, 1], F32, name="z_sb")

            for _rep in range(reps):
                # ------------- front: loads, hi/lo U/V build, centroid -------
                nc.sync.dma_start(out=xtf, in_=xt[:, :])
                nc.scalar.dma_start(out=xb_s, in_=xb[:, :])
                nc.scalar.dma_start(out=mk_s, in_=mk[:, :])
                nc.gpsimd.dma_start(out=w0_s, in_=w0a[:, :])
                nc.gpsimd.dma_start(out=w1_s, in_=w1a[:, :])
                nc.gpsimd.dma_start(out=w2_s, in_=w2a[:, :])
                nc.gpsimd.dma_start(out=wz_s, in_=wz2[:, :])
                nc.gpsimd.dma_start(out=bz_s, in_=bzt[:, :])
                nc.vector.memset(onesb2, 1.0)
                # rows 9..31 stay zero and contribute nothing to the K=36 matmul
                nc.vector.memset(U13, 0.0)
                nc.vector.memset(V13, 0.0)
                nc.gpsimd.memset(ones31, 1.0)
                nc.gpsimd.memset(ones128, 1.0)
                nc.gpsimd.memset(ones1r, 1.0)
                # hi/lo split of the coordinates
                nc.vector.tensor_copy(out=xh3, in_=xtf)
                nc.vector.tensor_tensor(out=xl3, in0=xtf, in1=xh3, op=OP.subtract)
                nc.scalar.activation(out=xsq, in_=xtf, func=AF.Square)

                with tc.tile_pool(name="fpsum", bufs=1, space="PSUM") as fp:
                    sqp = fp.tile([1, N], F32, name="sqp")
                    for g in range(4):
                        nc.tensor.matmul(
                            sqp[:, 512 * g:512 * (g + 1)], lhsT=ones31,
                            rhs=xsq[:, 512 * g:512 * (g + 1)], start=True, stop=True,
                        )
                    nc.vector.tensor_scalar_mul(out=sneg, in0=sqp, scalar1=-0.5)
                    nc.vector.tensor_copy(out=shl, in_=sneg)
                    nc.vector.tensor_tensor(out=sll, in0=sneg, in1=shl, op=OP.subtract)
                    # engines only address partition starts {0,32,64,96}; DMA
                    # places single rows at arbitrary partitions.
                    # U rows: xh xh xl | -sqh/2 -sql/2 | 1 1
                    # V rows: xh xl xh |   1     1     | -sqh/2 -sql/2
                    nc.scalar.dma_start(out=U13[0:3, :], in_=xh3)
                    nc.scalar.dma_start(out=U13[3:6, :], in_=xh3)
                    nc.scalar.dma_start(out=U13[6:9, :], in_=xl3)
                    nc.scalar.dma_start(out=U13[32:33, :], in_=shl)
                    nc.scalar.dma_start(out=U13[33:34, :], in_=sll)
                    nc.scalar.dma_start(out=U13[34:36, :], in_=onesb2)
                    nc.sync.dma_start(out=V13[0:3, :], in_=xh3)
                    nc.sync.dma_start(out=V13[3:6, :], in_=xl3)
                    nc.sync.dma_start(out=V13[6:9, :], in_=xh3)
                    nc.sync.dma_start(out=V13[32:34, :], in_=onesb2)
                    nc.sync.dma_start(out=V13[34:35, :], in_=shl)
                    nc.sync.dma_start(out=V13[35:36, :], in_=sll)

                    # centroid = sum(x*m)/max(count,1); count = sum(m)
                    nc.vector.reduce_sum(out=mkr, in_=mk_s, axis=mybir.AxisListType.X)
                    cntp = fp.tile([1, 1], F32, name="cntp")
                    nc.tensor.matmul(cntp, lhsT=mkr, rhs=ones128, start=True, stop=True)
                    nc.vector.tensor_scalar_max(out=cnt_sb, in0=cntp, scalar1=1.0)
                    nc.vector.reciprocal(out=invc1, in_=cnt_sb)
                    invb = fp.tile([P, 1], F32, name="invb")
                    nc.tensor.matmul(invb, lhsT=ones1r, rhs=invc1, start=True, stop=True)
                    nc.vector.tensor_copy(out=invc_sb, in_=invb)
                    nc.vector.tensor_scalar_mul(out=msc, in0=mk_s, scalar1=invc_sb)
                    nc.vector.tensor_copy(out=msc_b, in_=msc)
                    centp = fp.tile([D, 1], F32, name="centp")
                    for r in range(R):
                        nc.tensor.matmul(
                            centp, lhsT=xb_s[:, D * r:D * (r + 1)], rhs=msc[:, r:r + 1],
                            start=(r == 0), stop=(r == R - 1),
                        )
                    nc.vector.tensor_copy(out=cent_sb, in_=centp)
                    nc.gpsimd.dma_start(out=cent_d[:, :], in_=cent_sb)
                    cent_ap = cent_d[:, :]
                    cbc = bass.AP(
                        tensor=cent_ap.tensor, offset=cent_ap.offset,
                        ap=[[0, 1], [0, R], [1, D]],
                    )
                    nc.gpsimd.dma_start(out=crow, in_=cbc)
                    c48p = fp.tile([P, R * D], F32, name="c48p")
                    nc.tensor.matmul(c48p, lhsT=ones1r, rhs=crow, start=True, stop=True)
                    nc.vector.tensor_tensor(out=h0, in0=xb_s, in1=c48p, op=OP.subtract)

                # ------------- phases 1+2 share the PSUM budget --------------
                with tc.tile_pool(name="spsum", bufs=2, space="PSUM") as sp, \
                     tc.tile_pool(name="pal0", bufs=1, space="PSUM") as pl0:
                    # phase 1: E = exp(-dist), 16 row-blocks of [128, 2048].
                    # Layer-0's (E @ h0) aggregation rides along: its four
                    # 512-col groups live at partition offsets 32g of ONE
                    # psum bank (d_in=3), so each E tile is consumed by PE
                    # right after its exp, hidden under the ACT-bound phase.
                    pa0 = pl0.tile([P, 512], F32, name="pa0")
                    for i in range(R):
                        for t in range(2):
                            ps = sp.tile([P, 1024], F32, name="ps", tag="ps")
                            for gg in range(2):
                                j0 = 1024 * t + 512 * gg
                                nc.tensor.matmul(
                                    ps[:, 512 * gg:512 * (gg + 1)],
                                    lhsT=U13[0:36, P * i:P * (i + 1)],
                                    rhs=V13[0:36, j0:j0 + 512],
                                    start=True, stop=True,
                                )
                            nc.scalar.activation(
                                out=E_all[:, N * i + 1024 * t: N * i + 1024 * (t + 1)],
                                in_=ps, func=AF.Exp, scale=2.0,
                            )
                        for g in range(4):
                            nc.tensor.matmul(
                                pa0[32 * g:32 * g + D, :],
                                lhsT=h0[:, D * i:D * (i + 1)],
                                rhs=E_all[:, N * i + 512 * g: N * i + 512 * (g + 1)],
                                start=(i == 0), stop=(i == R - 1),
                                tile_position=(0, 32 * g),
                            )
                    for g in range(4):
                        nc.vector.tensor_copy(
                            out=EhT[:D, 512 * g:512 * (g + 1)],
                            in_=pa0[32 * g:32 * g + D, :],
                        )

                with tc.tile_pool(name="apsum", bufs=3, space="PSUM") as apl, \
                     tc.tile_pool(name="bpsum", bufs=3, space="PSUM") as bpl:

                    # phase 2: three message-passing layers
                    layers = [
                        (h0, D, w0_s, HID[0], h1),
                        (h1, HID[0], w1_s, HID[1], h2),
                        (h2, HID[1], w2_s, HID[2], h3),
                    ]
                    for li, (hin, d_in, w_s, d_out, hout) in enumerate(layers):
                        # (E @ h)^T accumulated over the 16 point-chunks
                        # (layer 0's aggregation already ran under phase 1)
                        for g in range(4) if li > 0 else ():
                            pa = apl.tile([P, 512], F32, name="pa", tag="pa")
                            for r in range(R):
                                nc.tensor.matmul(
                                    pa[:d_in, :], lhsT=hin[:, d_in * r:d_in * (r + 1)],
                                    rhs=E_all[:, N * r + 512 * g: N * r + 512 * (g + 1)],
                                    start=(r == 0), stop=(r == R - 1),
                                )
                            nc.vector.tensor_copy(
                                out=EhT[:d_in, 512 * g:512 * (g + 1)], in_=pa[:d_in, :]
                            )
                        # @W_aug, center, variance, rsqrt, scale, swish
                        for half in range(2):
                            for c in range(8 * half, 8 * half + 8):
                                pb = bpl.tile([P, d_out + 1], F32, name="pb", tag="pb")
                                ehc = EhT[:d_in, P * c:P * (c + 1)]
                                nc.tensor.matmul(
                                    pb, lhsT=ehc, rhs=w_s[:, 0:d_out + 1],
                                    start=True, stop=False,
                                )
                                nc.tensor.matmul(
                                    pb, lhsT=ehc,
                                    rhs=w_s[:, d_out + 1:2 * (d_out + 1)],
                                    start=False, stop=True,
                                )
                                ysl = y_all[:, d_out * c:d_out * (c + 1)]
                                # y0 = u - mean(u)   (psum col d_out holds -mean)
                                nc.vector.tensor_scalar(
                                    out=ysl, in0=pb[:, :d_out],
                                    scalar1=pb[:, d_out:d_out + 1], scalar2=None,
                                    op0=OP.add,
                                )
                                sqo = scr.tile([P, d_out], F32, name="sqo", tag="sqo")
                                nc.scalar.activation(
                                    out=sqo, in_=ysl, func=AF.Square,
                                    accum_out=varN[:, c:c + 1],
                                )
                            h8 = slice(8 * half, 8 * half + 8)
                            _emit_rsqrt(
                                nc, rstd[:, h8], varN[:, h8], rs_w, rs_t1, d_out
                            )
                            for c in range(8 * half, 8 * half + 8):
                                ysl = y_all[:, d_out * c:d_out * (c + 1)]
                                nc.vector.tensor_scalar_mul(
                                    out=ysl, in0=ysl, scalar1=rstd[:, c:c + 1]
                                )
                            yhalf = y_all[:, d_out * 8 * half:d_out * 8 * (half + 1)]
                            hhalf = hout[:, d_out * 8 * half:d_out * 8 * (half + 1)]
                            nc.scalar.activation(out=hhalf, in_=yhalf, func=AF.Silu)

                # ------------- phase 3: masked mean pool + readout -----------
                with tc.tile_pool(name="tpsum", bufs=1, space="PSUM") as tp:
                    gf0 = tp.tile([P, 1], F32, name="gf0")
                    gf1 = tp.tile([P, 1], F32, name="gf1")
                    for t, gft in enumerate((gf0, gf1)):
                        for c in range(R):
                            o = HID[2] * c + P * t
                            nc.tensor.matmul(
                                gft, lhsT=h3[:, o:o + P], rhs=msc_b[:, c:c + 1],
                                start=(c == 0), stop=(c == R - 1),
                            )
                    nc.vector.tensor_copy(out=gf_b[:, 0:1], in_=gf0)
                    nc.vector.tensor_copy(out=gf_b[:, 1:2], in_=gf1)
                    nc.vector.tensor_tensor(out=gfl_b[:, 0:1], in0=gf0,
                                            in1=gf_b[:, 0:1], op=OP.subtract)
                    nc.vector.tensor_tensor(out=gfl_b[:, 1:2], in0=gf1,
                                            in1=gf_b[:, 1:2], op=OP.subtract)
                    zps = tp.tile([P, 1], F32, name="zps")
                    # wz_s columns: [wzh half0 | wzh half1 | wzl half0 | wzl half1]
                    # z ~= Wzh.gfh + Wzl.gfh + Wzh.gfl   (drop Wzl.gfl)
                    zmm = [(0, gf_b, 0), (1, gf_b, 1), (2, gf_b, 0), (3, gf_b, 1),
                           (0, gfl_b, 0), (1, gfl_b, 1)]
                    for k, (wcol, gsrc, gcol) in enumerate(zmm):
                        nc.tensor.matmul(
                            zps, lhsT=wz_s[:, LAT * wcol:LAT * (wcol + 1)],
                            rhs=gsrc[:, gcol:gcol + 1],
                            start=(k == 0), stop=(k == len(zmm) - 1),
                        )
                    nc.vector.scalar_tensor_tensor(
                        out=z_sb, in0=zps, scalar=1.0, in1=bz_s,
                        op0=OP.mult, op1=OP.add,
                    )
                    nc.sync.dma_start(out=zout[:, :], in_=z_sb)
    return nc


_NC_CACHE = None


def _get_nc():
    global _NC_CACHE
    if _NC_CACHE is None:
        _apply_tile_patch()
        nc = _build()
        nc.finalize()   # Bacc.compile(): wait legalization + register alloc
        _NC_CACHE = nc
    return _NC_CACHE


def _host_prep(inputs):
    x = np.asarray(inputs["x"], np.float32)
    mask = np.asarray(inputs["mask"], np.float32)
    W = [np.asarray(inputs[f"W{i}"], np.float32) for i in range(3)]
    Wz = np.asarray(inputs["Wz"], np.float32)
    bz = np.asarray(inputs["bz"], np.float32)

    def hilo(a):
        hi = a.astype(ml_dtypes.bfloat16)
        lo = (a - hi.astype(np.float32)).astype(ml_dtypes.bfloat16)
        return hi, lo

    waug = []
    for i in range(3):
        a = np.concatenate([W[i], -W[i].mean(axis=1, keepdims=True)], axis=1)
        hi, lo = hilo(a)
        waug.append(np.ascontiguousarray(np.concatenate([hi, lo], axis=1)))
    wzflat = np.concatenate([Wz[:P, :], Wz[P:, :]], axis=1)
    wzh, wzl = hilo(wzflat)
    wz2 = np.ascontiguousarray(np.concatenate([wzh, wzl], axis=1))
    bzr = np.ascontiguousarray(bz.reshape(P, 1))

    in_maps = []
    for bi in range(B):
        in_maps.append({
            "xt": np.ascontiguousarray(
                x[bi].reshape(P, R, D).transpose(2, 1, 0).reshape(D, N)
            ),
            "xb": np.ascontiguousarray(x[bi].reshape(P, R * D)),
            "mk": np.ascontiguousarray(mask[bi].reshape(P, R)),
            "w0a": waug[0], "w1a": waug[1], "w2a": waug[2],
            "wz2": wz2, "bzt": bzr,
        })
    return in_maps


def kernel(**inputs):
    for i in range(3):
        if (np.any(np.asarray(inputs[f"b{i}"])) or
                np.any(np.asarray(inputs[f"be{i}"])) or
                np.any(np.asarray(inputs[f"g{i}"]) != 1.0)):
            raise NotImplementedError(
                "kernel specialized for zero LN/layer biases and unit gains"
            )
    in_maps = _host_prep(inputs)
    nc = _get_nc()
    res = run_bass_kernel_spmd(nc, in_maps, core_ids=list(range(B)))
    return np.stack([res.results[i]["z"][:, 0] for i in range(B)]).astype(np.float32)


# revision 22
# speedup vs baseline: 1.8009x; 1.3447x over previous
"""Trainium2 Bass kernel for nn_EquivariantDeepSetsEncoder.

Strategy: data-parallel over batch (B=8) across 8 NeuronCores; one batch per
core. Per core the full 2048x2048 attention matrix E = exp(-pairwise_dist)
stays resident in SBUF (8 MB in bf16) and is reused by all three
message-passing layers, so HBM traffic is just the tiny inputs/outputs.

Key algebraic simplifications (exact, not approximations):
  * pairwise distances depend only on coordinate differences, so the centroid
    subtraction cancels inside the attention; E is built from raw x.
  * -dist(i,j) = x_i.x_j - |x_i|^2/2 - |x_j|^2/2 (then exp with scale=2),
    which a single augmented matmul computes.
  * softmax row-normalization (1/rowsum) cancels through LayerNorm's scale
    invariance because the pre-LN bias b_i is zero, so it is never computed.
  * LayerNorm mean comes for free as an extra (negated, pre-averaged) column
    of the layer weight matrix.

Precision: fp32 matmuls stream at 4 cycles/column on the PE, bf16 at 1. The
attention logits need ~fp32 accuracy, so they use a hi/lo split-precision
bf16 matmul (x = xh + xl, keeping the xh*xh + xh*xl + xl*xh terms, same for
the squared norms; K grows 5 -> 13 but moving cycles drop 4x). E, h and W
are plain bf16 (their quantization error averages out over 2048 points);
LayerNorm statistics and activations are computed in fp32.
"""

import math
import os

import numpy as np
import ml_dtypes

import concourse.bass as bass
import concourse.bacc as bacc
import concourse.mybir as mybir
import concourse.tile as tile
from concourse.bass_utils import run_bass_kernel_spmd
from concourse.vector_clock import ScopedClock

F32 = mybir.dt.float32
BF16 = mybir.dt.bfloat16
U32 = mybir.dt.uint32
AF = mybir.ActivationFunctionType
OP = mybir.AluOpType

B, N, D = 8, 2048, 3
P, R = 128, 16          # N = P * R; point (p, r) = original index 16*p + r
HID = (64, 128, 256)
LAT = 128
EPS = 1e-6
RSQRT_MAGIC = 0x5F3759DF


# ---------------------------------------------------------------------------
# Workaround for a walrus codegen limit in this toolchain: a NO_STRUCT
# instruction (Drain) can carry at most one sync-wait command. Tile's exit
# path attaches the full global-clock wait set to a single drain; split the
# waits across several drains instead. (Bacc's later legalization handles
# the rest of the instructions.)
def _split_drain_and_barrier(self, tick_clock, wait_clock):
    nc = self.nc
    drain_inst = nc.sync.drain()
    wait_clock.add_sem_waits(
        drain_inst.ins, ScopedClock({None: tick_clock.global_clock})
    )
    si = drain_inst.ins.sync_info
    waits = list(si.on_wait) if si is not None else []
    if len(waits) > 1:
        si.on_wait = [waits[0]]
        for w in waits[1:]:
            d2 = nc.sync.drain()
            if d2.ins.sync_info is not None:
                d2.ins.sync_info.on_wait = [w]
            else:
                d2.ins.sync_info = mybir.SyncInfo(on_wait=[w], on_update=[])
    nc.all_engine_barrier()
    assert self.sems is not None
    popped = nc._tile_sem_poison_stack.pop()
    assert popped is self._sem_poison
    nc.clear_and_free_semaphores(list(self.sems.allocated().values()))
    nc.all_engine_barrier()


def _apply_tile_patch():
    if os.environ.get("NO_DRAIN_PATCH", "0") == "1":
        return
    tile.TileContext._drain_and_barrier = _split_drain_and_barrier


# ---------------------------------------------------------------------------
def _emit_rsqrt(nc, out_ap, var_ap, w_t, t1_t, d_out):
    """out = sqrt(d_out / (var_ap + d_out*EPS)) == 1/sqrt(var + EPS), where
    var_ap holds sum-of-squares (d_out * var). Fast-inverse-sqrt seed plus
    three Newton iterations, fp32-accurate; DVE only (no activation table)."""
    nc.vector.tensor_single_scalar(out=w_t, in_=var_ap, scalar=d_out * EPS, op=OP.add)
    w_u = w_t.bitcast(U32)
    t1_u = t1_t.bitcast(U32)
    # seed bits = MAGIC - (w_bits >> 1). The DVE ALU is fp32 for +/-, so the
    # subtraction happens in float on the integer VALUES (result stays in
    # [5e8, 1.6e9], no wrap) and the uint32 output cast restores the bits;
    # the ~1e-5 relative bit noise is far below the seed's 3.4% error.
    nc.vector.tensor_scalar(
        out=t1_u, in0=w_u, scalar1=1, scalar2=None, op0=OP.logical_shift_right,
    )
    r_t = out_ap
    r_u = r_t.bitcast(U32)
    nc.vector.tensor_scalar(
        out=r_u, in0=t1_u, scalar1=-1.0, scalar2=float(RSQRT_MAGIC),
        op0=OP.mult, op1=OP.add,
    )
    sqd = math.sqrt(float(d_out))
    for it in range(2):
        nc.vector.tensor_tensor(out=t1_t, in0=r_t, in1=r_t, op=OP.mult)
        nc.vector.tensor_tensor(out=t1_t, in0=t1_t, in1=w_t, op=OP.mult)
        nc.vector.tensor_scalar(
            out=t1_t, in0=t1_t, scalar1=-0.5, scalar2=1.5, op0=OP.mult, op1=OP.add
        )
        if it < 1:
            nc.vector.tensor_tensor(out=r_t, in0=r_t, in1=t1_t, op=OP.mult)
        else:
            # fold the sqrt(d_out) factor into the final Newton multiply
            nc.vector.scalar_tensor_tensor(
                out=r_t, in0=r_t, scalar=sqd, in1=t1_t, op0=OP.mult, op1=OP.mult
            )


def _build(reps=1):
    nc = bacc.Bacc()
    xt = nc.dram_tensor("xt", [D, N], F32, kind="ExternalInput")
    xb = nc.dram_tensor("xb", [P, R * D], F32, kind="ExternalInput")
    mk = nc.dram_tensor("mk", [P, R], F32, kind="ExternalInput")
    w0a = nc.dram_tensor("w0a", [D, 2 * (HID[0] + 1)], BF16, kind="ExternalInput")
    w1a = nc.dram_tensor("w1a", [HID[0], 2 * (HID[1] + 1)], BF16, kind="ExternalInput")
    w2a = nc.dram_tensor("w2a", [HID[1], 2 * (HID[2] + 1)], BF16, kind="ExternalInput")
    wz2 = nc.dram_tensor("wz2", [P, 4 * LAT], BF16, kind="ExternalInput")
    bzt = nc.dram_tensor("bzt", [P, 1], F32, kind="ExternalInput")
    zout = nc.dram_tensor("z", [P, 1], F32, kind="ExternalOutput")
    cent_d = nc.dram_tensor("cent_scratch", [1, D], F32, kind="Internal")

    with tile.TileContext(nc) as tc:
        with tc.tile_pool(name="persist", bufs=1) as pp, \
             tc.tile_pool(name="scr", bufs=2) as scr:
            E_all = pp.tile([P, R * N], BF16, name="E_all")
            U13 = pp.tile([36, N], BF16, name="U13")
            V13 = pp.tile([36, N], BF16, name="V13")
            xtf = pp.tile([D, N], F32, name="xtf")
            xh3 = pp.tile([D, N], BF16, name="xh3")
            xl3 = pp.tile([D, N], BF16, name="xl3")
            xsq = pp.tile([D, N], F32, name="xsq")
            sneg = pp.tile([1, N], F32, name="sneg")
            shl = pp.tile([1, N], BF16, name="shl")
            sll = pp.tile([1, N], BF16, name="sll")
            onesb2 = pp.tile([2, N], BF16, name="onesb2")
            h0 = pp.tile([P, R * D], BF16, name="h0")
            h1 = pp.tile([P, R * HID[0]], BF16, name="h1")
            h2 = pp.tile([P, R * HID[1]], BF16, name="h2")
            h3 = pp.tile([P, R * HID[2]], BF16, name="h3")
            y_all = pp.tile([P, R * HID[2]], F32, name="y_all")
            EhT = pp.tile([P, N], BF16, name="EhT")
            xb_s = pp.tile([P, R * D], F32, name="xb_s")
            mk_s = pp.tile([P, R], F32, name="mk_s")
            msc = pp.tile([P, R], F32, name="msc")
            msc_b = pp.tile([P, R], BF16, name="msc_b")
            crow = pp.tile([1, R * D], F32, name="crow")
            w0_s = pp.tile([D, 2 * (HID[0] + 1)], BF16, name="w0_s")
            w1_s = pp.tile([HID[0], 2 * (HID[1] + 1)], BF16, name="w1_s")
            w2_s = pp.tile([HID[1], 2 * (HID[2] + 1)], BF16, name="w2_s")
            wz_s = pp.tile([P, 4 * LAT], BF16, name="wz_s")
            gfl_b = pp.tile([P, 2], BF16, name="gfl_b")
            bz_s = pp.tile([P, 1], F32, name="bz_s")
            ones31 = pp.tile([D, 1], F32, name="ones31")
            ones128 = pp.tile([P, 1], F32, name="ones128")
            ones1r = pp.tile([1, P], F32, name="ones1r")
            mkr = pp.tile([P, 1], F32, name="mkr")
            cnt_sb = pp.tile([1, 1], F32, name="cnt_sb")
            invc1 = pp.tile([1, 1], F32, name="invc1")
            invc_sb = pp.tile([P, 1], F32, name="invc_sb")
            cent_sb = pp.tile([D, 1], F32, name="cent_sb")
            varN = pp.tile([P, R], F32, name="varN")
            rstd = pp.tile([P, R], F32, name="rstd")
            rs_w = pp.tile([P, 8], F32, name="rs_w")
            rs_t1 = pp.tile([P, 8], F32, name="rs_t1")
            gf_b = pp.tile([P, 2], BF16, name="gf_b")
            z_sb = pp.tile([P, 1], F32, name="z_sb")

            for _rep in range(reps):
                # ------------- front: loads, hi/lo U/V build, centroid -------
                nc.sync.dma_start(out=xtf, in_=xt[:, :])
                nc.scalar.dma_start(out=xb_s, in_=xb[:, :])
                nc.scalar.dma_start(out=mk_s, in_=mk[:, :])
                nc.gpsimd.dma_start(out=w0_s, in_=w0a[:, :])
                nc.gpsimd.dma_start(out=w1_s, in_=w1a[:, :])
                nc.gpsimd.dma_start(out=w2_s, in_=w2a[:, :])
                nc.gpsimd.dma_start(out=wz_s, in_=wz2[:, :])
                nc.gpsimd.dma_start(out=bz_s, in_=bzt[:, :])
                nc.vector.memset(onesb2, 1.0)
                # rows 9..31 stay zero and contribute nothing to the K=36 matmul
                nc.vector.memset(U13, 0.0)
                nc.vector.memset(V13, 0.0)
                nc.gpsimd.memset(ones31, 1.0)
                nc.gpsimd.memset(ones128, 1.0)
                nc.gpsimd.memset(ones1r, 1.0)
                # dummy exp: pulls the ~2.7us exp table load into the
                # front's DMA window instead of stalling phase 1
                warm = pp.tile([1, 1], F32, name="warm")
                nc.vector.memset(warm, 0.0)
                nc.scalar.activation(out=warm, in_=warm, func=AF.Exp)
                # hi/lo split of the coordinates
                nc.vector.tensor_copy(out=xh3, in_=xtf)
                nc.vector.tensor_tensor(out=xl3, in0=xtf, in1=xh3, op=OP.subtract)
                nc.scalar.activation(out=xsq, in_=xtf, func=AF.Square)

                with tc.tile_pool(name="fpsum", bufs=1, space="PSUM") as fp:
                    sqp = fp.tile([1, N], F32, name="sqp")
                    for g in range(4):
                        nc.tensor.matmul(
                            sqp[:, 512 * g:512 * (g + 1)], lhsT=ones31,
                            rhs=xsq[:, 512 * g:512 * (g + 1)], start=True, stop=True,
                        )
                    nc.vector.tensor_scalar_mul(out=sneg, in0=sqp, scalar1=-0.5)
                    nc.vector.tensor_copy(out=shl, in_=sneg)
                    nc.vector.tensor_tensor(out=sll, in0=sneg, in1=shl, op=OP.subtract)
                    # engines only address partition starts {0,32,64,96}; DMA
                    # places single rows at arbitrary partitions.
                    # U rows: xh xh xl | -sqh/2 -sql/2 | 1 1
                    # V rows: xh xl xh |   1     1     | -sqh/2 -sql/2
                    nc.scalar.dma_start(out=U13[0:3, :], in_=xh3)
                    nc.scalar.dma_start(out=U13[3:6, :], in_=xh3)
                    nc.scalar.dma_start(out=U13[6:9, :], in_=xl3)
                    nc.scalar.dma_start(out=U13[32:33, :], in_=shl)
                    nc.scalar.dma_start(out=U13[33:34, :], in_=sll)
                    nc.scalar.dma_start(out=U13[34:36, :], in_=onesb2)
                    nc.sync.dma_start(out=V13[0:3, :], in_=xh3)
                    nc.sync.dma_start(out=V13[3:6, :], in_=xl3)
                    nc.sync.dma_start(out=V13[6:9, :], in_=xh3)
                    nc.sync.dma_start(out=V13[32:34, :], in_=onesb2)
                    nc.sync.dma_start(out=V13[34:35, :], in_=shl)
                    nc.sync.dma_start(out=V13[35:36, :], in_=sll)

                    # centroid = sum(x*m)/max(count,1); count = sum(m)
                    nc.vector.reduce_sum(out=mkr, in_=mk_s, axis=mybir.AxisListType.X)
                    cntp = fp.tile([1, 1], F32, name="cntp")
                    nc.tensor.matmul(cntp, lhsT=mkr, rhs=ones128, start=True, stop=True)
                    nc.vector.tensor_scalar_max(out=cnt_sb, in0=cntp, scalar1=1.0)
                    nc.vector.reciprocal(out=invc1, in_=cnt_sb)
                    invb = fp.tile([P, 1], F32, name="invb")
                    nc.tensor.matmul(invb, lhsT=ones1r, rhs=invc1, start=True, stop=True)
                    nc.vector.tensor_copy(out=invc_sb, in_=invb)
                    nc.vector.tensor_scalar_mul(out=msc, in0=mk_s, scalar1=invc_sb)
                    nc.vector.tensor_copy(out=msc_b, in_=msc)
                    centp = fp.tile([D, 1], F32, name="centp")
                    for r in range(R):
                        nc.tensor.matmul(
                            centp, lhsT=xb_s[:, D * r:D * (r + 1)], rhs=msc[:, r:r + 1],
                            start=(r == 0), stop=(r == R - 1),
                        )
                    nc.vector.tensor_copy(out=cent_sb, in_=centp)
                    nc.gpsimd.dma_start(out=cent_d[:, :], in_=cent_sb)
                    cent_ap = cent_d[:, :]
                    cbc = bass.AP(
                        tensor=cent_ap.tensor, offset=cent_ap.offset,
                        ap=[[0, 1], [0, R], [1, D]],
                    )
                    nc.gpsimd.dma_start(out=crow, in_=cbc)
                    c48p = fp.tile([P, R * D], F32, name="c48p")
                    nc.tensor.matmul(c48p, lhsT=ones1r, rhs=crow, start=True, stop=True)
                    nc.vector.tensor_tensor(out=h0, in0=xb_s, in1=c48p, op=OP.subtract)

                # ------------- phases 1+2 share the PSUM budget --------------
                with tc.tile_pool(name="spsum", bufs=2, space="PSUM") as sp, \
                     tc.tile_pool(name="pal0", bufs=1, space="PSUM") as pl0:
                    # phase 1: E = exp(-dist), 16 row-blocks of [128, 2048].
                    # Layer-0's (E @ h0) aggregation rides along: its four
                    # 512-col groups live at partition offsets 32g of ONE
                    # psum bank (d_in=3), so each E tile is consumed by PE
                    # right after its exp, hidden under the ACT-bound phase.
                    pa0 = pl0.tile([P, 512], F32, name="pa0")
                    for i in range(R):
                        for t in range(2):
                            ps = sp.tile([P, 1024], F32, name="ps", tag="ps")
                            for gg in range(2):
                                j0 = 1024 * t + 512 * gg
                                nc.tensor.matmul(
                                    ps[:, 512 * gg:512 * (gg + 1)],
                                    lhsT=U13[0:36, P * i:P * (i + 1)],
                                    rhs=V13[0:36, j0:j0 + 512],
                                    start=True, stop=True,
                                )
                            nc.scalar.activation(
                                out=E_all[:, N * i + 1024 * t: N * i + 1024 * (t + 1)],
                                in_=ps, func=AF.Exp, scale=2.0,
                            )
                        for g in range(4):
                            nc.tensor.matmul(
                                pa0[32 * g:32 * g + D, :],
                                lhsT=h0[:, D * i:D * (i + 1)],
                                rhs=E_all[:, N * i + 512 * g: N * i + 512 * (g + 1)],
                                start=(i == 0), stop=(i == R - 1),
                                tile_position=(0, 32 * g),
                            )
                    for g in range(4):
                        nc.vector.tensor_copy(
                            out=EhT[:D, 512 * g:512 * (g + 1)],
                            in_=pa0[32 * g:32 * g + D, :],
                        )

                with tc.tile_pool(name="apsum", bufs=3, space="PSUM") as apl, \
                     tc.tile_pool(name="bpsum", bufs=3, space="PSUM") as bpl:

                    # phase 2: three message-passing layers
                    layers = [
                        (h0, D, w0_s, HID[0], h1),
                        (h1, HID[0], w1_s, HID[1], h2),
                        (h2, HID[1], w2_s, HID[2], h3),
                    ]
                    for li, (hin, d_in, w_s, d_out, hout) in enumerate(layers):
                        # (E @ h)^T accumulated over the 16 point-chunks
                        # (layer 0's aggregation already ran under phase 1)
                        for g in range(4) if li > 0 else ():
                            pa = apl.tile([P, 512], F32, name="pa", tag="pa")
                            for r in range(R):
                                nc.tensor.matmul(
                                    pa[:d_in, :], lhsT=hin[:, d_in * r:d_in * (r + 1)],
                                    rhs=E_all[:, N * r + 512 * g: N * r + 512 * (g + 1)],
                                    start=(r == 0), stop=(r == R - 1),
                                )
                            nc.vector.tensor_copy(
                                out=EhT[:d_in, 512 * g:512 * (g + 1)], in_=pa[:d_in, :]
                            )
                        # @W_aug, center, variance, rsqrt, scale, swish
                        for half in range(2):
                            for c in range(8 * half, 8 * half + 8):
                                pb = bpl.tile([P, d_out + 1], F32, name="pb", tag="pb")
                                ehc = EhT[:d_in, P * c:P * (c + 1)]
                                nc.tensor.matmul(
                                    pb, lhsT=ehc, rhs=w_s[:, 0:d_out + 1],
                                    start=True, stop=False,
                                )
                                nc.tensor.matmul(
                                    pb, lhsT=ehc,
                                    rhs=w_s[:, d_out + 1:2 * (d_out + 1)],
                                    start=False, stop=True,
                                )
                                ysl = y_all[:, d_out * c:d_out * (c + 1)]
                                # y0 = u - mean(u)   (psum col d_out holds -mean)
                                nc.vector.tensor_scalar(
                                    out=ysl, in0=pb[:, :d_out],
                                    scalar1=pb[:, d_out:d_out + 1], scalar2=None,
                                    op0=OP.add,
                                )
                                sqo = scr.tile([P, d_out], F32, name="sqo", tag="sqo")
                                nc.scalar.activation(
                                    out=sqo, in_=ysl, func=AF.Square,
                                    accum_out=varN[:, c:c + 1],
                                )
                            h8 = slice(8 * half, 8 * half + 8)
                            _emit_rsqrt(
                                nc, rstd[:, h8], varN[:, h8], rs_w, rs_t1, d_out
                            )
                            for c in range(8 * half, 8 * half + 8):
                                ysl = y_all[:, d_out * c:d_out * (c + 1)]
                                nc.vector.tensor_scalar_mul(
                                    out=ysl, in0=ysl, scalar1=rstd[:, c:c + 1]
                                )
                            yhalf = y_all[:, d_out * 8 * half:d_out * 8 * (half + 1)]
                            hhalf = hout[:, d_out * 8 * half:d_out * 8 * (half + 1)]
                            nc.scalar.activation(out=hhalf, in_=yhalf, func=AF.Silu)

                # ------------- phase 3: masked mean pool + readout -----------
                with tc.tile_pool(name="tpsum", bufs=1, space="PSUM") as tp:
                    gf0 = tp.tile([P, 1], F32, name="gf0")
                    gf1 = tp.tile([P, 1], F32, name="gf1")
                    for t, gft in enumerate((gf0, gf1)):
                        for c in range(R):
                            o = HID[2] * c + P * t
                            nc.tensor.matmul(
                                gft, lhsT=h3[:, o:o + P], rhs=msc_b[:, c:c + 1],
                                start=(c == 0), stop=(c == R - 1),
                            )
                    nc.vector.tensor_copy(out=gf_b[:, 0:1], in_=gf0)
                    nc.vector.tensor_copy(out=gf_b[:, 1:2], in_=gf1)
                    nc.vector.tensor_tensor(out=gfl_b[:, 0:1], in0=gf0,
                                            in1=gf_b[:, 0:1], op=OP.subtract)
                    nc.vector.tensor_tensor(out=gfl_b[:, 1:2], in0=gf1,
                                            in1=gf_b[:, 1:2], op=OP.subtract)
                    zps = tp.tile([P, 1], F32, name="zps")
                    # wz_s columns: [wzh half0 | wzh half1 | wzl half0 | wzl half1]
                    # z ~= Wzh.gfh + Wzl.gfh + Wzh.gfl   (drop Wzl.gfl)
                    zmm = [(0, gf_b, 0), (1, gf_b, 1), (2, gf_b, 0), (3, gf_b, 1),
                           (0, gfl_b, 0), (1, gfl_b, 1)]
                    for k, (wcol, gsrc, gcol) in enumerate(zmm):
                        nc.tensor.matmul(
                            zps, lhsT=wz_s[:, LAT * wcol:LAT * (wcol + 1)],
                            rhs=gsrc[:, gcol:gcol + 1],
                            start=(k == 0), stop=(k == len(zmm) - 1),
                        )
                    nc.vector.scalar_tensor_tensor(
                        out=z_sb, in0=zps, scalar=1.0, in1=bz_s,
                        op0=OP.mult, op1=OP.add,
                    )
                    nc.sync.dma_start(out=zout[:, :], in_=z_sb)
    return nc


_NC_CACHE = None


def _get_nc():
    global _NC_CACHE
    if _NC_CACHE is None:
        _apply_tile_patch()
        nc = _build()
        nc.finalize()   # Bacc.compile(): wait legalization + register alloc
        _NC_CACHE = nc
    return _NC_CACHE


def _host_prep(inputs):
    x = np.asarray(inputs["x"], np.float32)
    mask = np.asarray(inputs["mask"], np.float32)
    W = [np.asarray(inputs[f"W{i}"], np.float32) for i in range(3)]
    Wz = np.asarray(inputs["Wz"], np.float32)
    bz = np.asarray(inputs["bz"], np.float32)

    def hilo(a):
        hi = a.astype(ml_dtypes.bfloat16)
        lo = (a - hi.astype(np.float32)).astype(ml_dtypes.bfloat16)
        return hi, lo

    waug = []
    for i in range(3):
        a = np.concatenate([W[i], -W[i].mean(axis=1, keepdims=True)], axis=1)
        hi, lo = hilo(a)
        waug.append(np.ascontiguousarray(np.concatenate([hi, lo], axis=1)))
    wzflat = np.concatenate([Wz[:P, :], Wz[P:, :]], axis=1)
    wzh, wzl = hilo(wzflat)
    wz2 = np.ascontiguousarray(np.concatenate([wzh, wzl], axis=1))
    bzr = np.ascontiguousarray(bz.reshape(P, 1))

    in_maps = []
    for bi in range(B):
        in_maps.append({
            "xt": np.ascontiguousarray(
                x[bi].reshape(P, R, D).transpose(2, 1, 0).reshape(D, N)
            ),
            "xb": np.ascontiguousarray(x[bi].reshape(P, R * D)),
            "mk": np.ascontiguousarray(mask[bi].reshape(P, R)),
            "w0a": waug[0], "w1a": waug[1], "w2a": waug[2],
            "wz2": wz2, "bzt": bzr,
        })
    return in_maps


def kernel(**inputs):
    for i in range(3):
        if (np.any(np.asarray(inputs[f"b{i}"])) or
                np.any(np.asarray(inputs[f"be{i}"])) or
                np.any(np.asarray(inputs[f"g{i}"]) != 1.0)):
            raise NotImplementedError(
                "kernel specialized for zero LN/layer biases and unit gains"
            )
    in_maps = _host_prep(inputs)
    nc = _get_nc()
    res = run_bass_kernel_spmd(nc, in_maps, core_ids=list(range(B)))
    return np.stack([res.results[i]["z"][:, 0] for i in range(B)]).astype(np.float32)
